# revision 13
# baseline (speedup 1.0000x reference)
"""CSWM transition GNN kernel for 8 TRN2 NeuronCores.

Sharding: data-parallel over the 512 edge-groups (the quirky edge list is
block-diagonal over groups of 15 consecutive flat rows). Each core gets
64 groups (960 edge rows) + 64 of the 512 zero-agg tail rows = 1024 node
rows. No cross-core communication.

Host-side algebra:
  - cat(xi,xi,xj)@e_w0 = xi@(W0a+W0b) + xj@W0c          (per-node U,V)
  - final edge matmul commutes with scatter-add; W2 then folds into the
    node MLP first layer: nw0s = e_w2 @ n_w0[532:1556]
  - per-edge work: one 1024x1024 fp8 matmul + LayerNorm + relu

v2 structure:
  - packed-210 edge slots (no diagonal): slot (g,i,j') -> edge
    (i, (i+1+j') mod 15) via an overlapping-window AP on a duplicated V
  - r-build adds on gpsimd, relus split scalar/vector
  - all transposes via DMA xbar (dma_start_transpose), none on PE
  - weights stored pre-transposed in DRAM (contiguous DMA)
  - node L2 in fp8 DoubleRow (hT evicted as fp8)
"""

import numpy as np
import ml_dtypes
import bass_rust

import concourse.bass as bass
import concourse.mybir as mybir
import concourse.tile as tile
from concourse import bacc
from concourse.bass_utils import run_bass_kernel_spmd

BF16 = mybir.dt.bfloat16
F32 = mybir.dt.float32
F8 = mybir.dt.float8e4
DR = mybir.MatmulPerfMode.DoubleRow
AF = mybir.ActivationFunctionType
ALU = mybir.AluOpType

P = 128
D = 512            # embedding dim
H = 1024           # hidden dim
A_DIM = 20         # action dim
B = 512            # batch
K = 16             # objects
NG = 512           # total edge groups (block-diag over 15-row groups)
N_CORES = 8
G_CORE = NG // N_CORES          # 64 groups per core
EDGE_ROWS = G_CORE * 15         # 960
EXTRA_ROWS = (B * K - NG * 15) // N_CORES   # 64 zero-agg tail rows per core
N_ROWS = EDGE_ROWS + EXTRA_ROWS  # 1024 node rows per core
GB = 8                          # groups per aggregation block
NBLK = G_CORE // GB             # 8 blocks per core
E_BLK = GB * 210                # 1680 edges per block (diagonal-free)
NCHUNK = (E_BLK + P - 1) // P   # 14 chunks of 128 edge-slots
NODES_BLK = GB * 15             # 120
EPS = 1e-5
FP8_H = True                    # node hidden in fp8 -> L2 DoubleRow

# r-build engine split: which fs-chunks' (u+v) adds go on gpsimd vs vector,
# and relus on scalar vs vector/gpsimd.
ADD_ENG = ['g', 'g', 'g', 'g', 'g', 'g', 'v', 'v']
RELU_ENG = ['s', 's', 's', 's', 'g', 'g', 'v', 'v']


def _bf16(x):
    return np.ascontiguousarray(np.asarray(x, dtype=np.float32).astype(ml_dtypes.bfloat16))


def _f8(x):
    return np.ascontiguousarray(np.asarray(x, dtype=np.float32).astype(ml_dtypes.float8_e4m3))


def _f32(x):
    return np.ascontiguousarray(np.asarray(x, dtype=np.float32))


def _ap_window(full_ap, dims, extra_offset):
    """Raw AP with explicit (step, count) dims (supports overlap/broadcast)."""
    c = full_ap.copy()
    c.ap = bass_rust.VecI64Pair(dims)
    c.offset = c.offset + extra_offset
    return c


def _build_amat():
    """[NCHUNK, 128, 128] 0/1: slot g*210+i*14+j' -> node g*15+i (col),
    chunk-local rows; padding rows/cols zero."""
    a = np.zeros((NCHUNK * P, P), dtype=np.float32)
    for s in range(E_BLK):
        g, rem = divmod(s, 210)
        i = rem // 14
        a[s, g * 15 + i] = 1.0
    return a.reshape(NCHUNK, P, P)


def _build_program(trivial_affine_e: bool, trivial_affine_n: bool):
    nc = bacc.Bacc("TRN2", target_bir_lowering=False, debug=False)

    def din(name, shape, dt):
        return nc.declare_dram_parameter(name, list(shape), dt, isOutput=False)

    # all weight layouts are SBUF-ready: [P, k, n] contiguous
    xT = din("xT", (P, 4, N_ROWS), BF16)
    actT = din("actT", (A_DIM + 1, N_ROWS), BF16)
    wab = din("wab", (P, 4, H), BF16)
    w0c = din("w0c", (P, 4, H), BF16)
    b0 = din("b0", (P, 8), F32)
    w1 = din("w1", (P, 8, H), F8)
    b1 = din("b1", (1, H), F8)
    amat = din("amat", (P, NCHUNK, P), F8)
    nw0x = din("nw0x", (P, 4, H), BF16)
    nw0a = din("nw0a", (A_DIM + 1, H), BF16)
    nw0s = din("nw0s", (P, 8, H), BF16)
    nb0 = din("nb0", (P, 8), F32)
    nw1 = din("nw1", (P, 8, H), F8 if FP8_H else BF16)
    nb1 = din("nb1", (1, H), F32)
    nw2 = din("nw2", (P, 8, D), BF16)
    nb2 = din("nb2", (1, D), BF16)
    if not trivial_affine_e:
        e_g = din("e_g", (H,), F32)
        e_be = din("e_be", (H,), F32)
    if not trivial_affine_n:
        n_g = din("n_g", (H,), F32)
        n_be = din("n_be", (H,), F32)

    out = nc.declare_dram_parameter("out", [N_ROWS, D], F32, isOutput=True)

    with tile.TileContext(nc) as tc:
        with tc.tile_pool(name="const", bufs=1) as cpool:
            xT_s = cpool.tile([P, 4, N_ROWS], BF16)
            actT_s = cpool.tile([A_DIM + 1, N_ROWS], BF16)
            ones_row = cpool.tile([1, P], BF16)
            nc.vector.memset(ones_row[:], 1.0)
            eps_t = cpool.tile([P, 1], F32)
            nc.vector.memset(eps_t[:], EPS)
            # sT: aggregated-hidden, feature-major, block-slotted:
            # sT[p, k, blk*128 + node] = s[blk nodes' row, k*128+p]
            sT = cpool.tile([P, 8, NBLK * P], BF16)
            # s_blk: node-major aggregated hidden per block (dma-tp source);
            # rows 120..127 zeroed once (the dma transpose reads all 128).
            s_blks = [cpool.tile([P, H], BF16, tag=f"sblk{b}", name=f"s_blk{b}")
                      for b in range(NBLK)]
            for b in range(NBLK):
                nc.vector.memset(s_blks[b][96:P, :], 0.0)

            # ================= EDGE PHASE =================
            with (
                tc.tile_pool(name="ew", bufs=1) as ew,
                tc.tile_pool(name="uv", bufs=1) as uvp,
                tc.tile_pool(name="rp", bufs=2) as rp,
                tc.tile_pool(name="rb", bufs=5) as rbp,
                tc.tile_pool(name="zp", bufs=6) as zp,
                tc.tile_pool(name="st", bufs=4) as stp,
                tc.tile_pool(name="ps", bufs=3, space="PSUM") as ps,
                tc.tile_pool(name="pa", bufs=1, space="PSUM") as pa,
            ):
                wab_s = ew.tile([P, 4, H], BF16)
                w0c_s = ew.tile([P, 4, H], BF16)
                b0_t = ew.tile([P, 8], F32)
                nc.sync.dma_start(b0_t[:], b0[:])
                for ks in range(4):
                    nc.sync.dma_start(wab_s[:, ks, :], wab[:, ks, :])
                    nc.sync.dma_start(xT_s[:, ks, :], xT[:, ks, :])
                nc.sync.dma_start(actT_s[:], actT[:])
                for ks in range(4):
                    nc.sync.dma_start(w0c_s[:, ks, :], w0c[:, ks, :])
                w1_s = ew.tile([P, 8, H], F8)
                nc.sync.dma_start(w1_s[:], w1[:])
                amat_s = ew.tile([P, NCHUNK, P], F8)
                nc.sync.dma_start(amat_s[:], amat[:])
                b1_r = ew.tile([1, H], F8)
                nc.sync.dma_start(b1_r[:], b1[:])
                ones8 = ew.tile([1, P], F8)
                nc.vector.memset(ones8[:], 1.0)
                if not trivial_affine_e:
                    eg_b = ew.tile([P, H], F32)
                    nc.sync.dma_start(eg_b[:], e_g[None, :].to_broadcast((P, H)))
                    ebe_b = ew.tile([P, H], F32)
                    nc.sync.dma_start(ebe_b[:], e_be[None, :].to_broadcast((P, H)))

                # ---- U = x@(W0a+W0b)+b0 (feat-major), V duplicated [g]15+15 ----
                u_s = uvp.tile([P, 8, EDGE_ROWS], BF16, tag="u")
                v2_s = uvp.tile([P, 8, 2 * EDGE_ROWS], BF16, tag="v2")
                r_tiles = {}
                rb_tiles = {}

                def r_add(blk, fs):
                    """rb = U[i] + V[j] for block blk, feature-chunk fs."""
                    rb = rbp.tile([P, E_BLK], BF16, tag="rb", name=f"rb{blk}_{fs}")
                    rb_tiles[(blk, fs)] = rb
                    uap = _ap_window(
                        u_s[:], [[8 * EDGE_ROWS, P], [15, GB], [1, 15], [0, 14]],
                        fs * EDGE_ROWS + blk * NODES_BLK)
                    vap = _ap_window(
                        v2_s[:], [[16 * EDGE_ROWS, P], [30, GB], [1, 15], [1, 14]],
                        fs * 2 * EDGE_ROWS + blk * 2 * NODES_BLK + 1)
                    rb_o = rb[:].rearrange("p (g i j) -> p g i j", i=15, j=14)
                    eng = nc.gpsimd if ADD_ENG[fs] == 'g' else nc.vector
                    eng.tensor_tensor(rb_o, uap, vap, ALU.add)

                def r_relu(blk, fs):
                    rt = r_tiles[blk % 2]
                    rb = rb_tiles.pop((blk, fs))
                    e = RELU_ENG[fs]
                    if e == 's':
                        nc.scalar.activation(rt[:, fs, :], rb[:], AF.Relu)
                    elif e == 'v':
                        nc.vector.tensor_scalar_max(rt[:, fs, :], rb[:], 0.0)
                    else:
                        nc.gpsimd.tensor_scalar_max(rt[:, fs, :], rb[:], 0.0)

                for m in range(8):
                    for dst, wt, bias in ((u_s, wab_s, True), (v2_s, w0c_s, False)):
                        pt = ps.tile([P, H], F32, tag="mm")
                        for half, ncols in ((0, 512), (512, EDGE_ROWS - 512)):
                            for ks in range(4):
                                nc.tensor.matmul(
                                    pt[:, half:half + ncols],
                                    wt[:, ks, m * P:(m + 1) * P],
                                    xT_s[:, ks, half:half + ncols],
                                    start=(ks == 0), stop=(ks == 3),
                                )
                        if bias:
                            nc.scalar.activation(
                                dst[:, m, :], pt[:, :EDGE_ROWS], AF.Identity,
                                bias=b0_t[:, m:m + 1])
                        else:
                            # duplicated V: v2[g*30 + t] = v2[g*30+15+t] = V[g*15+t]
                            dvo = dst[:, m, :].rearrange("p (g t) -> p g t", t=30)
                            src = pt[:, :EDGE_ROWS].rearrange("p (g t) -> p g t", t=15)
                            nc.scalar.activation(dvo[:, :, 0:15], src, AF.Identity)
                            nc.vector.tensor_scalar_add(dvo[:, :, 15:30], src, 0.0)
                    if m < 2:
                        r_tiles[m] = rp.tile([P, 8, E_BLK], F8, tag="r", name=f"r_t{m}")
                    # build r for block 0 as soon as fs-chunk m of U/V lands
                    r_add(0, m)
                    r_relu(0, m)

                # ---- per-block: edge matmul + LN + aggregate ----
                def emit_agg_pair(pagg, cp, zpair):
                    lhs = amat_s[:, 2 * cp:2 * cp + 2, 0:NODES_BLK]
                    for half in (0, 512):
                        nc.tensor.matmul(pagg[:, half:half + 512], lhs,
                                         zpair[:, :, half:half + 512],
                                         start=(cp == 0), stop=(cp == NCHUNK // 2 - 1),
                                         perf_mode=DR)

                for blk in range(NBLK):
                    nxt = blk + 1
                    rt = r_tiles[blk % 2]
                    pagg = pa.tile([NODES_BLK, H], F32, tag="agg")
                    z_pairs = []
                    for et in range(NCHUNK):
                        m_sz = min(P, E_BLK - et * P)
                        pt = ps.tile([P, H], F32, tag="mm")
                        for kp in range(4):
                            lhs = rt[:, 2 * kp:2 * kp + 2, et * P:et * P + m_sz]
                            for half in (0, 512):
                                nc.tensor.matmul(pt[:m_sz, half:half + 512], lhs,
                                                 w1_s[:, 2 * kp:2 * kp + 2, half:half + 512],
                                                 start=(kp == 0), stop=False, perf_mode=DR)
                        for half in (0, 512):
                            nc.tensor.matmul(pt[:m_sz, half:half + 512], ones8[:, :m_sz],
                                             b1_r[:, half:half + 512], start=False, stop=True)

                        if et % 2 == 0:
                            z_pair = zp.tile([P, 2, H], F8, tag="z")
                            z_pairs.append(z_pair)
                        z_t = z_pairs[et // 2][:, et % 2, :]
                        if m_sz < P:
                            nc.vector.memset(z_pairs[et // 2][:, et % 2, :], 0.0)

                        # interleave next block's r-build with this block's chunks
                        if nxt < NBLK and et < 8:
                            r_add(nxt, et)

                        # LayerNorm stats on PSUM (pt already includes b1)
                        st6 = stp.tile([P, 12], F32, tag="st6")
                        nc.vector.bn_stats(st6[:m_sz, 0:6], pt[:m_sz, 0:512])
                        nc.vector.bn_stats(st6[:m_sz, 6:12], pt[:m_sz, 512:1024])
                        mv = stp.tile([P, 2], F32, tag="mv")
                        nc.vector.bn_aggr(mv[:m_sz], st6[:m_sz].rearrange("p (a b) -> p a b", b=6))
                        sc = stp.tile([P, 2], F32, tag="sc")
                        nc.scalar.activation(sc[:m_sz, 0:1], mv[:m_sz, 1:2],
                                             AF.Abs_reciprocal_sqrt, bias=eps_t[:m_sz])
                        nc.vector.tensor_scalar(sc[:m_sz, 1:2], mv[:m_sz, 0:1],
                                                sc[:m_sz, 0:1], -1.0,
                                                ALU.mult, ALU.mult)
                        if trivial_affine_e:
                            nc.scalar.activation(z_t[:m_sz], pt[:m_sz], AF.Relu,
                                                 bias=sc[:m_sz, 1:2], scale=sc[:m_sz, 0:1])
                        else:
                            zn = stp.tile([P, H], F32, tag="zn")
                            nc.scalar.activation(zn[:m_sz], pt[:m_sz], AF.Identity,
                                                 bias=sc[:m_sz, 1:2], scale=sc[:m_sz, 0:1])
                            nc.vector.tensor_tensor(zn[:m_sz], zn[:m_sz], eg_b[:m_sz], ALU.mult)
                            nc.vector.tensor_tensor(zn[:m_sz], zn[:m_sz], ebe_b[:m_sz], ALU.add)
                            nc.scalar.activation(z_t[:m_sz], zn[:m_sz], AF.Relu)

                        if nxt < NBLK and 4 <= et < 12:
                            r_relu(nxt, et - 4)

                        # aggregation trails the LN pipeline by one pair
                        if et % 2 == 1 and et >= 3:
                            emit_agg_pair(pagg, (et - 3) // 2, z_pairs[(et - 3) // 2])
                    emit_agg_pair(pagg, NCHUNK // 2 - 1, z_pairs[NCHUNK // 2 - 1])

                    # evict aggregated block; rows 120..127 are zeros
                    nc.scalar.activation(s_blks[blk][0:NODES_BLK, :], pagg[:], AF.Identity)
                    nc.scalar.dma_start_transpose(
                        sT[:, :, blk * P:(blk + 1) * P], s_blks[blk][:])

            # ================= NODE PHASE =================
            with (
                tc.tile_pool(name="nw", bufs=1) as nw,
                tc.tile_pool(name="nact", bufs=1) as na,
                tc.tile_pool(name="nst", bufs=3) as nst,
                tc.tile_pool(name="psA", bufs=2, space="PSUM") as psA,
                tc.tile_pool(name="psB", bufs=2, space="PSUM") as psB,
            ):
                nw0x_s = nw.tile([P, 4, H], BF16)
                nc.sync.dma_start(nw0x_s[:], nw0x[:])
                nw0a_s = nw.tile([A_DIM + 1, H], BF16)
                nc.sync.dma_start(nw0a_s[:], nw0a[:])
                nw0s_s = nw.tile([P, 8, H], BF16)
                nc.sync.dma_start(nw0s_s[:], nw0s[:])
                nw1_s = nw.tile([P, 8, H], F8 if FP8_H else BF16)
                nc.sync.dma_start(nw1_s[:], nw1[:])
                nw2_s = nw.tile([P, 8, D], BF16)
                nc.sync.dma_start(nw2_s[:], nw2[:])
                nb0_t = nw.tile([P, 8], F32)
                nc.sync.dma_start(nb0_t[:], nb0[:])
                nb1_b = nw.tile([P, H], F32)
                nc.sync.dma_start(nb1_b[:], nb1[:].to_broadcast((P, H)))
                nb2_s = nw.tile([1, D], BF16)
                nc.sync.dma_start(nb2_s[:], nb2[:])
                if not trivial_affine_n:
                    ng_b = nw.tile([P, H], F32)
                    nc.sync.dma_start(ng_b[:], n_g[None, :].to_broadcast((P, H)))
                    nbe_b = nw.tile([P, H], F32)
                    nc.sync.dma_start(nbe_b[:], n_be[None, :].to_broadcast((P, H)))

                sT_v = sT[:].rearrange("p k (b n) -> p k b n", n=P)

                # ---- node layer 1 -> hT (feat-major, relu+bias in evict) ----
                hT = na.tile([P, 8, N_ROWS], F8 if FP8_H else BF16, tag="hT")
                row_slices = [(0, 480, 0), (480, 480, 4), (960, 64, None)]
                for m in range(8):
                    msl = slice(m * P, (m + 1) * P)
                    for r0, nsz, sblk0 in row_slices:
                        pt = psB.tile([P, 512], F32, tag="l1")
                        rsl = slice(r0, r0 + nsz)
                        chunks = [(nw0x_s[:, ks, msl], xT_s[:, ks, rsl]) for ks in range(4)]
                        chunks.append((nw0a_s[:, msl], actT_s[:, rsl]))
                        if sblk0 is not None:
                            chunks += [(nw0s_s[:, ks, msl],
                                        sT_v[:, ks, sblk0:sblk0 + 4, 0:NODES_BLK])
                                       for ks in range(8)]
                        for ci, (lhs, rhs) in enumerate(chunks):
                            nc.tensor.matmul(pt[:, 0:nsz], lhs, rhs,
                                             start=(ci == 0), stop=(ci == len(chunks) - 1))
                        nc.scalar.activation(hT[:, m, rsl], pt[:, 0:nsz], AF.Relu,
                                             bias=nb0_t[:, m:m + 1])

                # ---- node layer 2 (row-major) + LN + relu -> z2, dma-tp ----
                z2T = na.tile([P, 8, N_ROWS], BF16, tag="z2T")
                for rt in range(8):
                    pt = psA.tile([P, H], F32, tag="mm")
                    if FP8_H:
                        for kp in range(4):
                            lhs = hT[:, 2 * kp:2 * kp + 2, rt * P:(rt + 1) * P]
                            for half in (0, 512):
                                nc.tensor.matmul(pt[:, half:half + 512], lhs,
                                                 nw1_s[:, 2 * kp:2 * kp + 2, half:half + 512],
                                                 start=(kp == 0), stop=(kp == 3), perf_mode=DR)
                    else:
                        for ks in range(8):
                            lhs = hT[:, ks, rt * P:(rt + 1) * P]
                            for half in (0, 512):
                                nc.tensor.matmul(pt[:, half:half + 512], lhs,
                                                 nw1_s[:, ks, half:half + 512],
                                                 start=(ks == 0), stop=(ks == 7))
                    h2b = nst.tile([P, H], F32, tag="h2b")
                    nc.vector.tensor_tensor(h2b[:], pt[:], nb1_b[:], ALU.add)
                    st6 = nst.tile([P, 12], F32, tag="st6")
                    nc.vector.bn_stats(st6[:, 0:6], h2b[:, 0:512])
                    nc.vector.bn_stats(st6[:, 6:12], h2b[:, 512:1024])
                    mv = nst.tile([P, 2], F32, tag="mv")
                    nc.vector.bn_aggr(mv[:], st6[:].rearrange("p (a b) -> p a b", b=6))
                    sc = nst.tile([P, 2], F32, tag="sc")
                    nc.scalar.activation(sc[:, 0:1], mv[:, 1:2],
                                         AF.Abs_reciprocal_sqrt, bias=eps_t[:])
                    nc.vector.tensor_scalar(sc[:, 1:2], mv[:, 0:1], sc[:, 0:1], -1.0,
                                            ALU.mult, ALU.mult)
                    z2 = nst.tile([P, H], BF16, tag="z2")
                    if trivial_affine_n:
                        nc.scalar.activation(z2[:], h2b[:], AF.Relu,
                                             bias=sc[:, 1:2], scale=sc[:, 0:1])
                    else:
                        zn = nst.tile([P, H], F32, tag="zn")
                        nc.scalar.activation(zn[:], h2b[:], AF.Identity,
                                             bias=sc[:, 1:2], scale=sc[:, 0:1])
                        nc.vector.tensor_tensor(zn[:], zn[:], ng_b[:], ALU.mult)
                        nc.vector.tensor_tensor(zn[:], zn[:], nbe_b[:], ALU.add)
                        nc.scalar.activation(z2[:], zn[:], AF.Relu)
                    nc.scalar.dma_start_transpose(z2T[:, :, rt * P:(rt + 1) * P], z2[:])

                # ---- node layer 3 + bias ----
                out_r = out[:].rearrange("(rt p) d -> p rt d", p=P)
                for rt in range(8):
                    pt = psB.tile([P, 512], F32, tag="l3")
                    for ks in range(8):
                        nc.tensor.matmul(pt[:, 0:D], z2T[:, ks, rt * P:(rt + 1) * P],
                                         nw2_s[:, ks, :], start=(ks == 0), stop=False)
                    nc.tensor.matmul(pt[:, 0:D], ones_row[:], nb2_s[:], start=False, stop=True)
                    outb = nst.tile([P, D], F32, tag="outb")
                    nc.scalar.activation(outb[:], pt[:, 0:D], AF.Identity)
                    nc.sync.dma_start(out_r[:, rt, :], outb[:])

    return nc


_PROG_CACHE = {}


def _get_program(trivial_e, trivial_n):
    key = (trivial_e, trivial_n, FP8_H)
    if key not in _PROG_CACHE:
        nc = _build_program(trivial_e, trivial_n)
        nc.finalize()
        _PROG_CACHE[key] = nc
    return _PROG_CACHE[key]


def _pkn(w, kt):
    """[K, N] -> [P, kt, N] (partition-major, SBUF-ready)."""
    return np.ascontiguousarray(w.reshape(kt, P, w.shape[1]).transpose(1, 0, 2))


def kernel(states, action, e_w0, e_b0, e_w1, e_b1, e_g, e_be, e_w2, e_b2,
           n_w0, n_b0, n_w1, n_b1, n_g, n_be, n_w2, n_b2):
    states = _f32(states)
    action = np.asarray(action).astype(np.int64)
    e_w0, e_b0, e_w1, e_b1 = _f32(e_w0), _f32(e_b0), _f32(e_w1), _f32(e_b1)
    e_g, e_be, e_w2, e_b2 = _f32(e_g), _f32(e_be), _f32(e_w2), _f32(e_b2)
    n_w0, n_b0, n_w1, n_b1 = _f32(n_w0), _f32(n_b0), _f32(n_w1), _f32(n_b1)
    n_g, n_be, n_w2, n_b2 = _f32(n_g), _f32(n_be), _f32(n_w2), _f32(n_b2)

    trivial_e = bool(np.all(e_g == 1.0) and np.all(e_be == 0.0))
    trivial_n = bool(np.all(n_g == 1.0) and np.all(n_be == 0.0))
    nc = _get_program(trivial_e, trivial_n)

    flat = states.reshape(-1, D)                        # [8192, 512]
    av = np.zeros((B, A_DIM * K), dtype=np.float32)
    av[np.arange(B), action] = 1.0
    av = av.reshape(-1, A_DIM)                          # [8192, 20]

    wab = e_w0[0:D] + e_w0[D:2 * D]                     # [512, 1024]
    w0c = e_w0[2 * D:3 * D]
    nw0x = n_w0[0:D]
    nw0a = n_w0[D:D + A_DIM]
    n_w0s_part = n_w0[D + A_DIM:]
    nw0s = e_w2 @ n_w0s_part                            # [1024, 1024]
    nw0a21 = np.concatenate([nw0a, (e_b2 @ n_w0s_part).reshape(1, H)], axis=0)

    amat = _build_amat()                                # [NCHUNK, 128, 128]
    amat_pkn = np.ascontiguousarray(amat.transpose(1, 0, 2))  # [P, NCHUNK, P]

    common = {
        "wab": _bf16(_pkn(wab, 4)), "w0c": _bf16(_pkn(w0c, 4)),
        "b0": _f32(e_b0.reshape(8, P).T), "w1": _f8(_pkn(e_w1, 8)),
        "b1": _f8(e_b1.reshape(1, H)),
        "amat": _f8(amat_pkn),
        "nw0x": _bf16(_pkn(nw0x, 4)), "nw0a": _bf16(nw0a21),
        "nw0s": _bf16(_pkn(nw0s, 8)), "nb0": _f32(n_b0.reshape(8, P).T),
        "nw1": (_f8 if FP8_H else _bf16)(_pkn(n_w1, 8)),
        "nb1": _f32(n_b1.reshape(1, H)),
        "nw2": _bf16(_pkn(n_w2, 8)), "nb2": _bf16(n_b2.reshape(1, D)),
    }
    if not trivial_e:
        common["e_g"] = _f32(e_g)
        common["e_be"] = _f32(e_be)
    if not trivial_n:
        common["n_g"] = _f32(n_g)
        common["n_be"] = _f32(n_be)

    in_maps = []
    row_idx = []
    for c in range(N_CORES):
        idx = np.concatenate([
            np.arange(c * EDGE_ROWS, (c + 1) * EDGE_ROWS),
            np.arange(NG * 15 + c * EXTRA_ROWS, NG * 15 + (c + 1) * EXTRA_ROWS),
        ])
        row_idx.append(idx)
        x_rows = flat[idx]                              # [1024, 512]
        xt = x_rows.T.reshape(4, P, N_ROWS).transpose(1, 0, 2)  # [P, 4, N]
        at = np.concatenate([av[idx].T, np.concatenate(
            [np.full((1, EDGE_ROWS), 14.0, np.float32),
             np.zeros((1, EXTRA_ROWS), np.float32)], axis=1)], axis=0)  # [21, 1024]
        m = dict(common)
        m["xT"] = _bf16(np.ascontiguousarray(xt))
        m["actT"] = _bf16(at)
        in_maps.append(m)

    res = run_bass_kernel_spmd(nc, in_maps, core_ids=list(range(N_CORES)))
    global LAST_RESULT
    LAST_RESULT = res

    out_full = np.empty((B * K, D), dtype=np.float32)
    for c in range(N_CORES):
        out_full[row_idx[c]] = flat[row_idx[c]] + res.results[c]["out"]
    return out_full.reshape(B, K, D)


# revision 14
# speedup vs baseline: 1.5624x; 1.5624x over previous
"""CSWM transition GNN kernel for 8 TRN2 NeuronCores.

Sharding: data-parallel over the 512 edge-groups (the quirky edge list is
block-diagonal over groups of 15 consecutive flat rows). Each core gets
64 groups (960 edge rows) + 64 of the 512 zero-agg tail rows = 1024 node
rows. No cross-core communication.

Host-side algebra:
  - cat(xi,xi,xj)@e_w0 = xi@(W0a+W0b) + xj@W0c          (per-node U,V)
  - final edge matmul commutes with scatter-add; W2 then folds into the
    node MLP first layer: nw0s = e_w2 @ n_w0[532:1556]
  - per-edge work: one 1024x1024 fp8 matmul + LayerNorm + relu

v2 structure:
  - packed-210 edge slots (no diagonal): slot (g,i,j') -> edge
    (i, (i+1+j') mod 15) via an overlapping-window AP on a duplicated V
  - r-build adds on gpsimd, relus split scalar/vector
  - all transposes via DMA xbar (dma_start_transpose), none on PE
  - weights stored pre-transposed in DRAM (contiguous DMA)
  - node L2 in fp8 DoubleRow (hT evicted as fp8)
"""

import numpy as np
import ml_dtypes
import bass_rust

import concourse.bass as bass
import concourse.mybir as mybir
import concourse.tile as tile
from concourse import bacc
from concourse.bass_utils import run_bass_kernel_spmd

BF16 = mybir.dt.bfloat16
F32 = mybir.dt.float32
F8 = mybir.dt.float8e4
DR = mybir.MatmulPerfMode.DoubleRow
AF = mybir.ActivationFunctionType
ALU = mybir.AluOpType

P = 128
D = 512            # embedding dim
H = 1024           # hidden dim
A_DIM = 20         # action dim
B = 512            # batch
K = 16             # objects
NG = 512           # total edge groups (block-diag over 15-row groups)
N_CORES = 8
G_CORE = NG // N_CORES          # 64 groups per core
EDGE_ROWS = G_CORE * 15         # 960
EXTRA_ROWS = (B * K - NG * 15) // N_CORES   # 64 zero-agg tail rows per core
N_ROWS = EDGE_ROWS + EXTRA_ROWS  # 1024 node rows per core
GB = 8                          # groups per aggregation block
NBLK = G_CORE // GB             # 8 blocks per core
E_BLK = GB * 210                # 1680 edges per block (diagonal-free)
NCHUNK = (E_BLK + P - 1) // P   # 14 chunks of 128 edge-slots
NODES_BLK = GB * 15             # 120
EPS = 1e-5
FP8_H = False                   # node hidden in fp8 -> L2 DoubleRow

# r-build engine split: adds on gpsimd (2-input tensor_tensor is ~2x DVE but
# runs on an otherwise idle engine); relu+fp8-cast must NOT go on gpsimd
# (its quantizing tensor_scalar path measured ~25us per op).
ADD_ENG = ['g', 'g', 'g', 'g', 'g', 'g', 'g', 'g']
RELU_ENG = ['s', 's', 's', 's', 's', 'v', 'v', 'v']


def _bf16(x):
    return np.ascontiguousarray(np.asarray(x, dtype=np.float32).astype(ml_dtypes.bfloat16))


def _f8(x):
    return np.ascontiguousarray(np.asarray(x, dtype=np.float32).astype(ml_dtypes.float8_e4m3))


def _f32(x):
    return np.ascontiguousarray(np.asarray(x, dtype=np.float32))


def _ap_window(full_ap, dims, extra_offset):
    """Raw AP with explicit (step, count) dims (supports overlap/broadcast)."""
    c = full_ap.copy()
    c.ap = bass_rust.VecI64Pair(dims)
    c.offset = c.offset + extra_offset
    return c


def _build_amat():
    """[NCHUNK, 128, 128] 0/1: slot g*210+i*14+j' -> node g*15+i (col),
    chunk-local rows; padding rows/cols zero."""
    a = np.zeros((NCHUNK * P, P), dtype=np.float32)
    for s in range(E_BLK):
        g, rem = divmod(s, 210)
        i = rem // 14
        a[s, g * 15 + i] = 1.0
    return a.reshape(NCHUNK, P, P)


def _build_program(trivial_affine_e: bool, trivial_affine_n: bool):
    nc = bacc.Bacc("TRN2", target_bir_lowering=False, debug=False)

    def din(name, shape, dt):
        return nc.declare_dram_parameter(name, list(shape), dt, isOutput=False)

    # all weight layouts are SBUF-ready: [P, k, n] contiguous
    xT = din("xT", (P, 4, N_ROWS), BF16)
    actT = din("actT", (A_DIM + 1, N_ROWS), BF16)
    wab = din("wab", (P, 4, H), BF16)
    w0c = din("w0c", (P, 4, H), BF16)
    b0 = din("b0", (P, 8), F32)
    w1 = din("w1", (P, 8, H), F8)
    b1 = din("b1", (1, H), F8)
    amat = din("amat", (P, NCHUNK, P), F8)
    nw0x = din("nw0x", (P, 4, H), BF16)
    nw0a = din("nw0a", (A_DIM + 1, H), BF16)
    nw0s = din("nw0s", (P, 8, H), BF16)
    nb0 = din("nb0", (P, 8), F32)
    nw1 = din("nw1", (P, 8, H), F8 if FP8_H else BF16)
    nb1 = din("nb1", (1, H), F32)
    nw2 = din("nw2", (P, 8, D), BF16)
    nb2 = din("nb2", (1, D), BF16)
    if not trivial_affine_e:
        e_g = din("e_g", (H,), F32)
        e_be = din("e_be", (H,), F32)
    if not trivial_affine_n:
        n_g = din("n_g", (H,), F32)
        n_be = din("n_be", (H,), F32)

    out = nc.declare_dram_parameter("out", [N_ROWS, D], F32, isOutput=True)

    with tile.TileContext(nc) as tc:
        with tc.tile_pool(name="const", bufs=1) as cpool:
            xT_s = cpool.tile([P, 4, N_ROWS], BF16)
            actT_s = cpool.tile([A_DIM + 1, N_ROWS], BF16)
            ones_row = cpool.tile([1, P], BF16)
            nc.vector.memset(ones_row[:], 1.0)
            eps_t = cpool.tile([P, 1], F32)
            nc.vector.memset(eps_t[:], EPS)
            # sT: aggregated-hidden, feature-major, block-slotted:
            # sT[p, k, blk*128 + node] = s[blk nodes' row, k*128+p]
            sT = cpool.tile([P, 8, NBLK * P], BF16)
            # s_blk: node-major aggregated hidden per block (dma-tp source);
            # rows 120..127 zeroed once (the dma transpose reads all 128).
            s_blks = [cpool.tile([P, H], BF16, tag=f"sblk{b}", name=f"s_blk{b}")
                      for b in range(NBLK)]
            for b in range(NBLK):
                nc.vector.memset(s_blks[b][96:P, :], 0.0)

            # ================= EDGE PHASE =================
            with (
                tc.tile_pool(name="ew", bufs=1) as ew,
                tc.tile_pool(name="uv", bufs=1) as uvp,
                tc.tile_pool(name="rp", bufs=2) as rp,
                tc.tile_pool(name="rb", bufs=5) as rbp,
                tc.tile_pool(name="zp", bufs=6) as zp,
                tc.tile_pool(name="st", bufs=4) as stp,
                tc.tile_pool(name="ps", bufs=3, space="PSUM") as ps,
                tc.tile_pool(name="pa", bufs=1, space="PSUM") as pa,
            ):
                wab_s = ew.tile([P, 4, H], BF16)
                w0c_s = ew.tile([P, 4, H], BF16)
                b0_t = ew.tile([P, 8], F32)
                nc.sync.dma_start(b0_t[:], b0[:])
                for ks in range(4):
                    nc.sync.dma_start(wab_s[:, ks, :], wab[:, ks, :])
                    nc.sync.dma_start(xT_s[:, ks, :], xT[:, ks, :])
                nc.sync.dma_start(actT_s[:], actT[:])
                for ks in range(4):
                    nc.sync.dma_start(w0c_s[:, ks, :], w0c[:, ks, :])
                w1_s = ew.tile([P, 8, H], F8)
                nc.sync.dma_start(w1_s[:], w1[:])
                amat_s = ew.tile([P, NCHUNK, P], F8)
                nc.sync.dma_start(amat_s[:], amat[:])
                b1_r = ew.tile([1, H], F8)
                nc.sync.dma_start(b1_r[:], b1[:])
                ones8 = ew.tile([1, P], F8)
                nc.vector.memset(ones8[:], 1.0)
                if not trivial_affine_e:
                    eg_b = ew.tile([P, H], F32)
                    nc.sync.dma_start(eg_b[:], e_g[None, :].to_broadcast((P, H)))
                    ebe_b = ew.tile([P, H], F32)
                    nc.sync.dma_start(ebe_b[:], e_be[None, :].to_broadcast((P, H)))

                # ---- U = x@(W0a+W0b)+b0 (feat-major), V duplicated [g]15+15 ----
                u_s = uvp.tile([P, 8, EDGE_ROWS], BF16, tag="u")
                v2_s = uvp.tile([P, 8, 2 * EDGE_ROWS], BF16, tag="v2")
                r_tiles = {}
                rb_tiles = {}

                def r_add(blk, fs):
                    """rb = U[i] + V[j] for block blk, feature-chunk fs."""
                    rb = rbp.tile([P, E_BLK], BF16, tag="rb", name=f"rb{blk}_{fs}")
                    rb_tiles[(blk, fs)] = rb
                    uap = _ap_window(
                        u_s[:], [[8 * EDGE_ROWS, P], [15, GB], [1, 15], [0, 14]],
                        fs * EDGE_ROWS + blk * NODES_BLK)
                    vap = _ap_window(
                        v2_s[:], [[16 * EDGE_ROWS, P], [30, GB], [1, 15], [1, 14]],
                        fs * 2 * EDGE_ROWS + blk * 2 * NODES_BLK + 1)
                    rb_o = rb[:].rearrange("p (g i j) -> p g i j", i=15, j=14)
                    eng = nc.gpsimd if ADD_ENG[fs] == 'g' else nc.vector
                    eng.tensor_tensor(rb_o, uap, vap, ALU.add)

                def r_relu(blk, fs):
                    rt = r_tiles[blk % 2]
                    rb = rb_tiles.pop((blk, fs))
                    e = RELU_ENG[fs]
                    if e == 's':
                        nc.scalar.activation(rt[:, fs, :], rb[:], AF.Relu)
                    elif e == 'v':
                        nc.vector.tensor_scalar_max(rt[:, fs, :], rb[:], 0.0)
                    else:
                        nc.gpsimd.tensor_scalar_max(rt[:, fs, :], rb[:], 0.0)

                for m in range(8):
                    for dst, wt, bias in ((u_s, wab_s, True), (v2_s, w0c_s, False)):
                        pt = ps.tile([P, H], F32, tag="mm")
                        for half, ncols in ((0, 512), (512, EDGE_ROWS - 512)):
                            for ks in range(4):
                                nc.tensor.matmul(
                                    pt[:, half:half + ncols],
                                    wt[:, ks, m * P:(m + 1) * P],
                                    xT_s[:, ks, half:half + ncols],
                                    start=(ks == 0), stop=(ks == 3),
                                )
                        if bias:
                            nc.scalar.activation(
                                dst[:, m, :], pt[:, :EDGE_ROWS], AF.Identity,
                                bias=b0_t[:, m:m + 1])
                        else:
                            # duplicated V: v2[g*30 + t] = v2[g*30+15+t] = V[g*15+t]
                            dvo = dst[:, m, :].rearrange("p (g t) -> p g t", t=30)
                            src = pt[:, :EDGE_ROWS].rearrange("p (g t) -> p g t", t=15)
                            nc.scalar.activation(dvo[:, :, 0:15], src, AF.Identity)
                            nc.vector.tensor_scalar_add(dvo[:, :, 15:30], src, 0.0)
                    if m < 2:
                        r_tiles[m] = rp.tile([P, 8, E_BLK], F8, tag="r", name=f"r_t{m}")
                    # build r for block 0 as soon as fs-chunk m of U/V lands
                    r_add(0, m)
                    r_relu(0, m)

                # ---- per-block: edge matmul + LN + aggregate ----
                def emit_agg_pair(pagg, cp, zpair):
                    lhs = amat_s[:, 2 * cp:2 * cp + 2, 0:NODES_BLK]
                    for half in (0, 512):
                        nc.tensor.matmul(pagg[:, half:half + 512], lhs,
                                         zpair[:, :, half:half + 512],
                                         start=(cp == 0), stop=(cp == NCHUNK // 2 - 1),
                                         perf_mode=DR)

                for blk in range(NBLK):
                    nxt = blk + 1
                    rt = r_tiles[blk % 2]
                    pagg = pa.tile([NODES_BLK, H], F32, tag="agg")
                    z_pairs = []
                    for et in range(NCHUNK):
                        m_sz = min(P, E_BLK - et * P)
                        pt = ps.tile([P, H], F32, tag="mm")
                        for kp in range(4):
                            lhs = rt[:, 2 * kp:2 * kp + 2, et * P:et * P + m_sz]
                            for half in (0, 512):
                                nc.tensor.matmul(pt[:m_sz, half:half + 512], lhs,
                                                 w1_s[:, 2 * kp:2 * kp + 2, half:half + 512],
                                                 start=(kp == 0), stop=False, perf_mode=DR)
                        for half in (0, 512):
                            nc.tensor.matmul(pt[:m_sz, half:half + 512], ones8[:, :m_sz],
                                             b1_r[:, half:half + 512], start=False, stop=True)

                        if et % 2 == 0:
                            z_pair = zp.tile([P, 2, H], F8, tag="z")
                            z_pairs.append(z_pair)
                        z_t = z_pairs[et // 2][:, et % 2, :]
                        if m_sz < P:
                            nc.vector.memset(z_pairs[et // 2][:, et % 2, :], 0.0)

                        # interleave next block's r-build with this block's chunks
                        if nxt < NBLK and et < 8:
                            r_add(nxt, et)

                        # LayerNorm stats on PSUM (pt already includes b1)
                        st6 = stp.tile([P, 12], F32, tag="st6")
                        nc.vector.bn_stats(st6[:m_sz, 0:6], pt[:m_sz, 0:512])
                        nc.vector.bn_stats(st6[:m_sz, 6:12], pt[:m_sz, 512:1024])
                        mv = stp.tile([P, 2], F32, tag="mv")
                        nc.vector.bn_aggr(mv[:m_sz], st6[:m_sz].rearrange("p (a b) -> p a b", b=6))
                        sc = stp.tile([P, 2], F32, tag="sc")
                        nc.scalar.activation(sc[:m_sz, 0:1], mv[:m_sz, 1:2],
                                             AF.Abs_reciprocal_sqrt, bias=eps_t[:m_sz])
                        nc.vector.tensor_scalar(sc[:m_sz, 1:2], mv[:m_sz, 0:1],
                                                sc[:m_sz, 0:1], -1.0,
                                                ALU.mult, ALU.mult)
                        if trivial_affine_e:
                            nc.scalar.activation(z_t[:m_sz], pt[:m_sz], AF.Relu,
                                                 bias=sc[:m_sz, 1:2], scale=sc[:m_sz, 0:1])
                        else:
                            zn = stp.tile([P, H], F32, tag="zn")
                            nc.scalar.activation(zn[:m_sz], pt[:m_sz], AF.Identity,
                                                 bias=sc[:m_sz, 1:2], scale=sc[:m_sz, 0:1])
                            nc.vector.tensor_tensor(zn[:m_sz], zn[:m_sz], eg_b[:m_sz], ALU.mult)
                            nc.vector.tensor_tensor(zn[:m_sz], zn[:m_sz], ebe_b[:m_sz], ALU.add)
                            nc.scalar.activation(z_t[:m_sz], zn[:m_sz], AF.Relu)

                        if nxt < NBLK and 4 <= et < 12:
                            r_relu(nxt, et - 4)

                        # aggregation trails the LN pipeline by one pair
                        if et % 2 == 1 and et >= 3:
                            emit_agg_pair(pagg, (et - 3) // 2, z_pairs[(et - 3) // 2])
                    emit_agg_pair(pagg, NCHUNK // 2 - 1, z_pairs[NCHUNK // 2 - 1])

                    # evict aggregated block; rows 120..127 are zeros
                    nc.scalar.activation(s_blks[blk][0:NODES_BLK, :], pagg[:], AF.Identity)
                    nc.scalar.dma_start_transpose(
                        sT[:, :, blk * P:(blk + 1) * P], s_blks[blk][:])

            # ================= NODE PHASE =================
            with (
                tc.tile_pool(name="nw", bufs=1) as nw,
                tc.tile_pool(name="nact", bufs=1) as na,
                tc.tile_pool(name="nst", bufs=3) as nst,
                tc.tile_pool(name="psA", bufs=2, space="PSUM") as psA,
                tc.tile_pool(name="psB", bufs=2, space="PSUM") as psB,
            ):
                nw0x_s = nw.tile([P, 4, H], BF16)
                nc.sync.dma_start(nw0x_s[:], nw0x[:])
                nw0a_s = nw.tile([A_DIM + 1, H], BF16)
                nc.sync.dma_start(nw0a_s[:], nw0a[:])
                nw0s_s = nw.tile([P, 8, H], BF16)
                nc.sync.dma_start(nw0s_s[:], nw0s[:])
                nw1_s = nw.tile([P, 8, H], F8 if FP8_H else BF16)
                nc.sync.dma_start(nw1_s[:], nw1[:])
                nw2_s = nw.tile([P, 8, D], BF16)
                nc.sync.dma_start(nw2_s[:], nw2[:])
                nb0_t = nw.tile([P, 8], F32)
                nc.sync.dma_start(nb0_t[:], nb0[:])
                nb1_b = nw.tile([P, H], F32)
                nc.sync.dma_start(nb1_b[:], nb1[:].to_broadcast((P, H)))
                nb2_s = nw.tile([1, D], BF16)
                nc.sync.dma_start(nb2_s[:], nb2[:])
                if not trivial_affine_n:
                    ng_b = nw.tile([P, H], F32)
                    nc.sync.dma_start(ng_b[:], n_g[None, :].to_broadcast((P, H)))
                    nbe_b = nw.tile([P, H], F32)
                    nc.sync.dma_start(nbe_b[:], n_be[None, :].to_broadcast((P, H)))

                sT_v = sT[:].rearrange("p k (b n) -> p k b n", n=P)

                # ---- node layer 1 -> hT (feat-major, relu+bias in evict) ----
                hT = na.tile([P, 8, N_ROWS], F8 if FP8_H else BF16, tag="hT")
                row_slices = [(0, 480, 0), (480, 480, 4), (960, 64, None)]
                for m in range(8):
                    msl = slice(m * P, (m + 1) * P)
                    for r0, nsz, sblk0 in row_slices:
                        pt = psB.tile([P, 512], F32, tag="l1")
                        rsl = slice(r0, r0 + nsz)
                        chunks = [(nw0x_s[:, ks, msl], xT_s[:, ks, rsl]) for ks in range(4)]
                        chunks.append((nw0a_s[:, msl], actT_s[:, rsl]))
                        if sblk0 is not None:
                            chunks += [(nw0s_s[:, ks, msl],
                                        sT_v[:, ks, sblk0:sblk0 + 4, 0:NODES_BLK])
                                       for ks in range(8)]
                        for ci, (lhs, rhs) in enumerate(chunks):
                            nc.tensor.matmul(pt[:, 0:nsz], lhs, rhs,
                                             start=(ci == 0), stop=(ci == len(chunks) - 1))
                        nc.scalar.activation(hT[:, m, rsl], pt[:, 0:nsz], AF.Relu,
                                             bias=nb0_t[:, m:m + 1])

                # ---- node layer 2 (row-major) + LN + relu -> z2, dma-tp ----
                z2T = na.tile([P, 8, N_ROWS], BF16, tag="z2T")
                for rt in range(8):
                    pt = psA.tile([P, H], F32, tag="mm")
                    if FP8_H:
                        for kp in range(4):
                            lhs = hT[:, 2 * kp:2 * kp + 2, rt * P:(rt + 1) * P]
                            for half in (0, 512):
                                nc.tensor.matmul(pt[:, half:half + 512], lhs,
                                                 nw1_s[:, 2 * kp:2 * kp + 2, half:half + 512],
                                                 start=(kp == 0), stop=(kp == 3), perf_mode=DR)
                    else:
                        for ks in range(8):
                            lhs = hT[:, ks, rt * P:(rt + 1) * P]
                            for half in (0, 512):
                                nc.tensor.matmul(pt[:, half:half + 512], lhs,
                                                 nw1_s[:, ks, half:half + 512],
                                                 start=(ks == 0), stop=(ks == 7))
                    h2b = nst.tile([P, H], F32, tag="h2b")
                    nc.vector.tensor_tensor(h2b[:], pt[:], nb1_b[:], ALU.add)
                    st6 = nst.tile([P, 12], F32, tag="st6")
                    nc.vector.bn_stats(st6[:, 0:6], h2b[:, 0:512])
                    nc.vector.bn_stats(st6[:, 6:12], h2b[:, 512:1024])
                    mv = nst.tile([P, 2], F32, tag="mv")
                    nc.vector.bn_aggr(mv[:], st6[:].rearrange("p (a b) -> p a b", b=6))
                    sc = nst.tile([P, 2], F32, tag="sc")
                    nc.scalar.activation(sc[:, 0:1], mv[:, 1:2],
                                         AF.Abs_reciprocal_sqrt, bias=eps_t[:])
                    nc.vector.tensor_scalar(sc[:, 1:2], mv[:, 0:1], sc[:, 0:1], -1.0,
                                            ALU.mult, ALU.mult)
                    z2 = nst.tile([P, H], BF16, tag="z2")
                    if trivial_affine_n:
                        nc.scalar.activation(z2[:], h2b[:], AF.Relu,
                                             bias=sc[:, 1:2], scale=sc[:, 0:1])
                    else:
                        zn = nst.tile([P, H], F32, tag="zn")
                        nc.scalar.activation(zn[:], h2b[:], AF.Identity,
                                             bias=sc[:, 1:2], scale=sc[:, 0:1])
                        nc.vector.tensor_tensor(zn[:], zn[:], ng_b[:], ALU.mult)
                        nc.vector.tensor_tensor(zn[:], zn[:], nbe_b[:], ALU.add)
                        nc.scalar.activation(z2[:], zn[:], AF.Relu)
                    nc.scalar.dma_start_transpose(z2T[:, :, rt * P:(rt + 1) * P], z2[:])

                # ---- node layer 3 + bias ----
                out_r = out[:].rearrange("(rt p) d -> p rt d", p=P)
                for rt in range(8):
                    pt = psB.tile([P, 512], F32, tag="l3")
                    for ks in range(8):
                        nc.tensor.matmul(pt[:, 0:D], z2T[:, ks, rt * P:(rt + 1) * P],
                                         nw2_s[:, ks, :], start=(ks == 0), stop=False)
                    nc.tensor.matmul(pt[:, 0:D], ones_row[:], nb2_s[:], start=False, stop=True)
                    outb = nst.tile([P, D], F32, tag="outb")
                    nc.scalar.activation(outb[:], pt[:, 0:D], AF.Identity)
                    nc.sync.dma_start(out_r[:, rt, :], outb[:])

    return nc


_PROG_CACHE = {}


def _get_program(trivial_e, trivial_n):
    key = (trivial_e, trivial_n, FP8_H)
    if key not in _PROG_CACHE:
        nc = _build_program(trivial_e, trivial_n)
        nc.finalize()
        _PROG_CACHE[key] = nc
    return _PROG_CACHE[key]


def _pkn(w, kt):
    """[K, N] -> [P, kt, N] (partition-major, SBUF-ready)."""
    return np.ascontiguousarray(w.reshape(kt, P, w.shape[1]).transpose(1, 0, 2))


def kernel(states, action, e_w0, e_b0, e_w1, e_b1, e_g, e_be, e_w2, e_b2,
           n_w0, n_b0, n_w1, n_b1, n_g, n_be, n_w2, n_b2):
    states = _f32(states)
    action = np.asarray(action).astype(np.int64)
    e_w0, e_b0, e_w1, e_b1 = _f32(e_w0), _f32(e_b0), _f32(e_w1), _f32(e_b1)
    e_g, e_be, e_w2, e_b2 = _f32(e_g), _f32(e_be), _f32(e_w2), _f32(e_b2)
    n_w0, n_b0, n_w1, n_b1 = _f32(n_w0), _f32(n_b0), _f32(n_w1), _f32(n_b1)
    n_g, n_be, n_w2, n_b2 = _f32(n_g), _f32(n_be), _f32(n_w2), _f32(n_b2)

    trivial_e = bool(np.all(e_g == 1.0) and np.all(e_be == 0.0))
    trivial_n = bool(np.all(n_g == 1.0) and np.all(n_be == 0.0))
    nc = _get_program(trivial_e, trivial_n)

    flat = states.reshape(-1, D)                        # [8192, 512]
    av = np.zeros((B, A_DIM * K), dtype=np.float32)
    av[np.arange(B), action] = 1.0
    av = av.reshape(-1, A_DIM)                          # [8192, 20]

    wab = e_w0[0:D] + e_w0[D:2 * D]                     # [512, 1024]
    w0c = e_w0[2 * D:3 * D]
    nw0x = n_w0[0:D]
    nw0a = n_w0[D:D + A_DIM]
    n_w0s_part = n_w0[D + A_DIM:]
    nw0s = e_w2 @ n_w0s_part                            # [1024, 1024]
    nw0a21 = np.concatenate([nw0a, (e_b2 @ n_w0s_part).reshape(1, H)], axis=0)

    amat = _build_amat()                                # [NCHUNK, 128, 128]
    amat_pkn = np.ascontiguousarray(amat.transpose(1, 0, 2))  # [P, NCHUNK, P]

    common = {
        "wab": _bf16(_pkn(wab, 4)), "w0c": _bf16(_pkn(w0c, 4)),
        "b0": _f32(e_b0.reshape(8, P).T), "w1": _f8(_pkn(e_w1, 8)),
        "b1": _f8(e_b1.reshape(1, H)),
        "amat": _f8(amat_pkn),
        "nw0x": _bf16(_pkn(nw0x, 4)), "nw0a": _bf16(nw0a21),
        "nw0s": _bf16(_pkn(nw0s, 8)), "nb0": _f32(n_b0.reshape(8, P).T),
        "nw1": (_f8 if FP8_H else _bf16)(_pkn(n_w1, 8)),
        "nb1": _f32(n_b1.reshape(1, H)),
        "nw2": _bf16(_pkn(n_w2, 8)), "nb2": _bf16(n_b2.reshape(1, D)),
    }
    if not trivial_e:
        common["e_g"] = _f32(e_g)
        common["e_be"] = _f32(e_be)
    if not trivial_n:
        common["n_g"] = _f32(n_g)
        common["n_be"] = _f32(n_be)

    in_maps = []
    row_idx = []
    for c in range(N_CORES):
        idx = np.concatenate([
            np.arange(c * EDGE_ROWS, (c + 1) * EDGE_ROWS),
            np.arange(NG * 15 + c * EXTRA_ROWS, NG * 15 + (c + 1) * EXTRA_ROWS),
        ])
        row_idx.append(idx)
        x_rows = flat[idx]                              # [1024, 512]
        xt = x_rows.T.reshape(4, P, N_ROWS).transpose(1, 0, 2)  # [P, 4, N]
        at = np.concatenate([av[idx].T, np.concatenate(
            [np.full((1, EDGE_ROWS), 14.0, np.float32),
             np.zeros((1, EXTRA_ROWS), np.float32)], axis=1)], axis=0)  # [21, 1024]
        m = dict(common)
        m["xT"] = _bf16(np.ascontiguousarray(xt))
        m["actT"] = _bf16(at)
        in_maps.append(m)

    res = run_bass_kernel_spmd(nc, in_maps, core_ids=list(range(N_CORES)))
    global LAST_RESULT
    LAST_RESULT = res

    out_full = np.empty((B * K, D), dtype=np.float32)
    for c in range(N_CORES):
        out_full[row_idx[c]] = flat[row_idx[c]] + res.results[c]["out"]
    return out_full.reshape(B, K, D)


# revision 15
# speedup vs baseline: 1.5970x; 1.0222x over previous
"""CSWM transition GNN kernel for 8 TRN2 NeuronCores.

Sharding: data-parallel over the 512 edge-groups (the quirky edge list is
block-diagonal over groups of 15 consecutive flat rows). Each core gets
64 groups (960 edge rows) + 64 of the 512 zero-agg tail rows = 1024 node
rows. No cross-core communication.

Host-side algebra:
  - cat(xi,xi,xj)@e_w0 = xi@(W0a+W0b) + xj@W0c          (per-node U,V)
  - final edge matmul commutes with scatter-add; W2 then folds into the
    node MLP first layer: nw0s = e_w2 @ n_w0[532:1556]
  - per-edge work: one 1024x1024 fp8 matmul + LayerNorm + relu

v2 structure:
  - packed-210 edge slots (no diagonal): slot (g,i,j') -> edge
    (i, (i+1+j') mod 15) via an overlapping-window AP on a duplicated V
  - r-build adds on gpsimd, relus split scalar/vector
  - all transposes via DMA xbar (dma_start_transpose), none on PE
  - weights stored pre-transposed in DRAM (contiguous DMA)
  - node L2 in fp8 DoubleRow (hT evicted as fp8)
"""

import numpy as np
import ml_dtypes
import bass_rust

import concourse.bass as bass
import concourse.mybir as mybir
import concourse.tile as tile
from concourse import bacc
from concourse.bass_utils import run_bass_kernel_spmd

BF16 = mybir.dt.bfloat16
F32 = mybir.dt.float32
F8 = mybir.dt.float8e4
DR = mybir.MatmulPerfMode.DoubleRow
AF = mybir.ActivationFunctionType
ALU = mybir.AluOpType

P = 128
D = 512            # embedding dim
H = 1024           # hidden dim
A_DIM = 20         # action dim
B = 512            # batch
K = 16             # objects
NG = 512           # total edge groups (block-diag over 15-row groups)
N_CORES = 8
G_CORE = NG // N_CORES          # 64 groups per core
EDGE_ROWS = G_CORE * 15         # 960
EXTRA_ROWS = (B * K - NG * 15) // N_CORES   # 64 zero-agg tail rows per core
N_ROWS = EDGE_ROWS + EXTRA_ROWS  # 1024 node rows per core
GB = 8                          # groups per aggregation block
NBLK = G_CORE // GB             # 8 blocks per core
E_BLK = GB * 210                # 1680 edges per block (diagonal-free)
NCHUNK = (E_BLK + P - 1) // P   # 14 chunks of 128 edge-slots
NODES_BLK = GB * 15             # 120
EPS = 1e-5
FP8_H = False                   # node hidden in fp8 -> L2 DoubleRow

# r-build engine split: adds on gpsimd (2-input tensor_tensor is ~2x DVE but
# runs on an otherwise idle engine); relu+fp8-cast must NOT go on gpsimd
# (its quantizing tensor_scalar path measured ~25us per op).
ADD_ENG = ['g', 'g', 'g', 'g', 'g', 'g', 'g', 'g']
RELU_ENG = ['s', 's', 's', 's', 's', 'v', 'v', 'v']


def _bf16(x):
    return np.ascontiguousarray(np.asarray(x, dtype=np.float32).astype(ml_dtypes.bfloat16))


def _f8(x):
    return np.ascontiguousarray(np.asarray(x, dtype=np.float32).astype(ml_dtypes.float8_e4m3))


def _f32(x):
    return np.ascontiguousarray(np.asarray(x, dtype=np.float32))


def _ap_window(full_ap, dims, extra_offset):
    """Raw AP with explicit (step, count) dims (supports overlap/broadcast)."""
    c = full_ap.copy()
    c.ap = bass_rust.VecI64Pair(dims)
    c.offset = c.offset + extra_offset
    return c


def _build_amat():
    """[NCHUNK, 128, 128] 0/1: slot g*210+i*14+j' -> node g*15+i (col),
    chunk-local rows; padding rows/cols zero."""
    a = np.zeros((NCHUNK * P, P), dtype=np.float32)
    for s in range(E_BLK):
        g, rem = divmod(s, 210)
        i = rem // 14
        a[s, g * 15 + i] = 1.0
    return a.reshape(NCHUNK, P, P)


def _build_program(trivial_affine_e: bool, trivial_affine_n: bool):
    nc = bacc.Bacc("TRN2", target_bir_lowering=False, debug=False)

    def din(name, shape, dt):
        return nc.declare_dram_parameter(name, list(shape), dt, isOutput=False)

    # all weight layouts are SBUF-ready: [P, k, n] contiguous
    xT = din("xT", (P, 4, N_ROWS), BF16)
    actT = din("actT", (A_DIM + 1, N_ROWS), BF16)
    wab = din("wab", (P, 4, H), BF16)
    w0c = din("w0c", (P, 4, H), BF16)
    b0 = din("b0", (P, 8), F32)
    w1 = din("w1", (P, 8, H), F8)
    b1 = din("b1", (1, H), F8)
    amat = din("amat", (P, NCHUNK, P), F8)
    nw0x = din("nw0x", (P, 4, H), BF16)
    nw0a = din("nw0a", (A_DIM + 1, H), BF16)
    nw0s = din("nw0s", (P, 8, H), BF16)
    nb0 = din("nb0", (P, 8), F32)
    nw1 = din("nw1", (P, 8, H), F8 if FP8_H else BF16)
    nb1 = din("nb1", (1, H), F32)
    nw2 = din("nw2", (P, 8, D), BF16)
    nb2 = din("nb2", (1, D), BF16)
    if not trivial_affine_e:
        e_g = din("e_g", (H,), F32)
        e_be = din("e_be", (H,), F32)
    if not trivial_affine_n:
        n_g = din("n_g", (H,), F32)
        n_be = din("n_be", (H,), F32)

    out = nc.declare_dram_parameter("out", [N_ROWS, D], F32, isOutput=True)

    with tile.TileContext(nc) as tc:
        with tc.tile_pool(name="const", bufs=1) as cpool:
            xT_s = cpool.tile([P, 4, N_ROWS], BF16)
            actT_s = cpool.tile([A_DIM + 1, N_ROWS], BF16)
            ones_row = cpool.tile([1, P], BF16)
            nc.vector.memset(ones_row[:], 1.0)
            eps_t = cpool.tile([P, 1], F32)
            nc.vector.memset(eps_t[:], EPS)
            # sT: aggregated-hidden, feature-major, block-slotted:
            # sT[p, k, blk*128 + node] = s[blk nodes' row, k*128+p]
            sT = cpool.tile([P, 8, NBLK * P], BF16)
            # s_blk: node-major aggregated hidden per block (dma-tp source);
            # rows 120..127 zeroed once (the dma transpose reads all 128).
            s_blks = [cpool.tile([P, H], BF16, tag=f"sblk{b}", name=f"s_blk{b}")
                      for b in range(NBLK)]
            for b in range(NBLK):
                nc.vector.memset(s_blks[b][96:P, :], 0.0)

            # ================= EDGE PHASE =================
            with (
                tc.tile_pool(name="ew", bufs=1) as ew,
                tc.tile_pool(name="uv", bufs=1) as uvp,
                tc.tile_pool(name="rp", bufs=3) as rp,
                tc.tile_pool(name="rb", bufs=8) as rbp,
                tc.tile_pool(name="zp", bufs=6) as zp,
                tc.tile_pool(name="st", bufs=4) as stp,
                tc.tile_pool(name="ps", bufs=3, space="PSUM") as ps,
                tc.tile_pool(name="pa", bufs=1, space="PSUM") as pa,
            ):
                wab_s = ew.tile([P, 4, H], BF16)
                w0c_s = ew.tile([P, 4, H], BF16)
                b0_t = ew.tile([P, 8], F32)
                nc.sync.dma_start(b0_t[:], b0[:])
                for ks in range(4):
                    nc.sync.dma_start(wab_s[:, ks, :], wab[:, ks, :])
                    nc.sync.dma_start(xT_s[:, ks, :], xT[:, ks, :])
                nc.sync.dma_start(actT_s[:], actT[:])
                for ks in range(4):
                    nc.sync.dma_start(w0c_s[:, ks, :], w0c[:, ks, :])
                w1_s = ew.tile([P, 8, H], F8)
                nc.sync.dma_start(w1_s[:], w1[:])
                amat_s = ew.tile([P, NCHUNK, P], F8)
                nc.sync.dma_start(amat_s[:], amat[:])
                b1_r = ew.tile([1, H], F8)
                nc.sync.dma_start(b1_r[:], b1[:])
                ones8 = ew.tile([1, P], F8)
                nc.vector.memset(ones8[:], 1.0)
                if not trivial_affine_e:
                    eg_b = ew.tile([P, H], F32)
                    nc.sync.dma_start(eg_b[:], e_g[None, :].to_broadcast((P, H)))
                    ebe_b = ew.tile([P, H], F32)
                    nc.sync.dma_start(ebe_b[:], e_be[None, :].to_broadcast((P, H)))

                # ---- U = x@(W0a+W0b)+b0 (feat-major), V duplicated [g]15+15 ----
                u_s = uvp.tile([P, 8, EDGE_ROWS], BF16, tag="u")
                v2_s = uvp.tile([P, 8, 2 * EDGE_ROWS], BF16, tag="v2")
                r_tiles = {}
                rb_tiles = {}

                def r_add(blk, fs):
                    """rb = U[i] + V[j] for block blk, feature-chunk fs."""
                    rb = rbp.tile([P, E_BLK], BF16, tag="rb", name=f"rb{blk}_{fs}")
                    rb_tiles[(blk, fs)] = rb
                    uap = _ap_window(
                        u_s[:], [[8 * EDGE_ROWS, P], [15, GB], [1, 15], [0, 14]],
                        fs * EDGE_ROWS + blk * NODES_BLK)
                    vap = _ap_window(
                        v2_s[:], [[16 * EDGE_ROWS, P], [30, GB], [1, 15], [1, 14]],
                        fs * 2 * EDGE_ROWS + blk * 2 * NODES_BLK + 1)
                    rb_o = rb[:].rearrange("p (g i j) -> p g i j", i=15, j=14)
                    eng = nc.gpsimd if ADD_ENG[fs] == 'g' else nc.vector
                    eng.tensor_tensor(rb_o, uap, vap, ALU.add)

                def r_relu(blk, fs):
                    rt = r_tiles[blk % 3]
                    rb = rb_tiles.pop((blk, fs))
                    e = RELU_ENG[fs]
                    if e == 's':
                        nc.scalar.activation(rt[:, fs, :], rb[:], AF.Relu)
                    elif e == 'v':
                        nc.vector.tensor_scalar_max(rt[:, fs, :], rb[:], 0.0)
                    else:
                        nc.gpsimd.tensor_scalar_max(rt[:, fs, :], rb[:], 0.0)

                for m in range(8):
                    for dst, wt, bias in ((u_s, wab_s, True), (v2_s, w0c_s, False)):
                        pt = ps.tile([P, H], F32, tag="mm")
                        for half, ncols in ((0, 512), (512, EDGE_ROWS - 512)):
                            for ks in range(4):
                                nc.tensor.matmul(
                                    pt[:, half:half + ncols],
                                    wt[:, ks, m * P:(m + 1) * P],
                                    xT_s[:, ks, half:half + ncols],
                                    start=(ks == 0), stop=(ks == 3),
                                )
                        if bias:
                            nc.scalar.activation(
                                dst[:, m, :], pt[:, :EDGE_ROWS], AF.Identity,
                                bias=b0_t[:, m:m + 1])
                        else:
                            # duplicated V: v2[g*30 + t] = v2[g*30+15+t] = V[g*15+t]
                            dvo = dst[:, m, :].rearrange("p (g t) -> p g t", t=30)
                            src = pt[:, :EDGE_ROWS].rearrange("p (g t) -> p g t", t=15)
                            nc.scalar.activation(dvo[:, :, 0:15], src, AF.Identity)
                            nc.vector.tensor_scalar_add(dvo[:, :, 15:30], src, 0.0)
                    if m < 3:
                        r_tiles[m] = rp.tile([P, 8, E_BLK], F8, tag="r", name=f"r_t{m}")
                    # build r for block 0 as soon as fs-chunk m of U/V lands
                    r_add(0, m)
                    r_relu(0, m)

                # ---- per-block: edge matmul + LN + aggregate ----
                def emit_agg_pair(pagg, cp, zpair):
                    lhs = amat_s[:, 2 * cp:2 * cp + 2, 0:NODES_BLK]
                    for half in (0, 512):
                        nc.tensor.matmul(pagg[:, half:half + 512], lhs,
                                         zpair[:, :, half:half + 512],
                                         start=(cp == 0), stop=(cp == NCHUNK // 2 - 1),
                                         perf_mode=DR)

                for blk in range(NBLK):
                    nxt = blk + 1
                    rt = r_tiles[blk % 3]
                    pagg = pa.tile([NODES_BLK, H], F32, tag="agg")
                    z_pairs = []
                    for et in range(NCHUNK):
                        m_sz = min(P, E_BLK - et * P)
                        pt = ps.tile([P, H], F32, tag="mm")
                        for kp in range(4):
                            lhs = rt[:, 2 * kp:2 * kp + 2, et * P:et * P + m_sz]
                            for half in (0, 512):
                                nc.tensor.matmul(pt[:m_sz, half:half + 512], lhs,
                                                 w1_s[:, 2 * kp:2 * kp + 2, half:half + 512],
                                                 start=(kp == 0), stop=False, perf_mode=DR)
                        for half in (0, 512):
                            nc.tensor.matmul(pt[:m_sz, half:half + 512], ones8[:, :m_sz],
                                             b1_r[:, half:half + 512], start=False, stop=True)

                        if et % 2 == 0:
                            z_pair = zp.tile([P, 2, H], F8, tag="z")
                            z_pairs.append(z_pair)
                        z_t = z_pairs[et // 2][:, et % 2, :]
                        if m_sz < P:
                            nc.vector.memset(z_pairs[et // 2][:, et % 2, :], 0.0)

                        # next block's adds all at chunk 0: gpsimd gets a
                        # full block period of lead time for its 8 serial adds
                        if nxt < NBLK and et == 0:
                            for fs in range(8):
                                r_add(nxt, fs)

                        # LayerNorm stats on PSUM (pt already includes b1)
                        st6 = stp.tile([P, 12], F32, tag="st6")
                        nc.vector.bn_stats(st6[:m_sz, 0:6], pt[:m_sz, 0:512])
                        nc.vector.bn_stats(st6[:m_sz, 6:12], pt[:m_sz, 512:1024])
                        mv = stp.tile([P, 2], F32, tag="mv")
                        nc.vector.bn_aggr(mv[:m_sz], st6[:m_sz].rearrange("p (a b) -> p a b", b=6))
                        sc = stp.tile([P, 2], F32, tag="sc")
                        nc.scalar.activation(sc[:m_sz, 0:1], mv[:m_sz, 1:2],
                                             AF.Abs_reciprocal_sqrt, bias=eps_t[:m_sz])
                        nc.vector.tensor_scalar(sc[:m_sz, 1:2], mv[:m_sz, 0:1],
                                                sc[:m_sz, 0:1], -1.0,
                                                ALU.mult, ALU.mult)
                        if trivial_affine_e:
                            nc.scalar.activation(z_t[:m_sz], pt[:m_sz], AF.Relu,
                                                 bias=sc[:m_sz, 1:2], scale=sc[:m_sz, 0:1])
                        else:
                            zn = stp.tile([P, H], F32, tag="zn")
                            nc.scalar.activation(zn[:m_sz], pt[:m_sz], AF.Identity,
                                                 bias=sc[:m_sz, 1:2], scale=sc[:m_sz, 0:1])
                            nc.vector.tensor_tensor(zn[:m_sz], zn[:m_sz], eg_b[:m_sz], ALU.mult)
                            nc.vector.tensor_tensor(zn[:m_sz], zn[:m_sz], ebe_b[:m_sz], ALU.add)
                            nc.scalar.activation(z_t[:m_sz], zn[:m_sz], AF.Relu)

                        if nxt < NBLK and 2 <= et < 10:
                            r_relu(nxt, et - 2)

                        # aggregation trails the LN pipeline by one pair
                        if et % 2 == 1 and et >= 3:
                            emit_agg_pair(pagg, (et - 3) // 2, z_pairs[(et - 3) // 2])
                    emit_agg_pair(pagg, NCHUNK // 2 - 1, z_pairs[NCHUNK // 2 - 1])

                    # evict aggregated block; rows 120..127 are zeros
                    nc.scalar.activation(s_blks[blk][0:NODES_BLK, :], pagg[:], AF.Identity)
                    nc.scalar.dma_start_transpose(
                        sT[:, :, blk * P:(blk + 1) * P], s_blks[blk][:])

            # ================= NODE PHASE =================
            with (
                tc.tile_pool(name="nw", bufs=1) as nw,
                tc.tile_pool(name="nact", bufs=1) as na,
                tc.tile_pool(name="nst", bufs=3) as nst,
                tc.tile_pool(name="psA", bufs=2, space="PSUM") as psA,
                tc.tile_pool(name="psB", bufs=2, space="PSUM") as psB,
            ):
                nw0x_s = nw.tile([P, 4, H], BF16)
                nc.sync.dma_start(nw0x_s[:], nw0x[:])
                nw0a_s = nw.tile([A_DIM + 1, H], BF16)
                nc.sync.dma_start(nw0a_s[:], nw0a[:])
                nw0s_s = nw.tile([P, 8, H], BF16)
                nc.sync.dma_start(nw0s_s[:], nw0s[:])
                nw1_s = nw.tile([P, 8, H], F8 if FP8_H else BF16)
                nc.sync.dma_start(nw1_s[:], nw1[:])
                nw2_s = nw.tile([P, 8, D], BF16)
                nc.sync.dma_start(nw2_s[:], nw2[:])
                nb0_t = nw.tile([P, 8], F32)
                nc.sync.dma_start(nb0_t[:], nb0[:])
                nb1_b = nw.tile([P, H], F32)
                nc.sync.dma_start(nb1_b[:], nb1[:].to_broadcast((P, H)))
                nb2_s = nw.tile([1, D], BF16)
                nc.sync.dma_start(nb2_s[:], nb2[:])
                if not trivial_affine_n:
                    ng_b = nw.tile([P, H], F32)
                    nc.sync.dma_start(ng_b[:], n_g[None, :].to_broadcast((P, H)))
                    nbe_b = nw.tile([P, H], F32)
                    nc.sync.dma_start(nbe_b[:], n_be[None, :].to_broadcast((P, H)))

                sT_v = sT[:].rearrange("p k (b n) -> p k b n", n=P)

                # ---- node layer 1 -> hT (feat-major, relu+bias in evict) ----
                hT = na.tile([P, 8, N_ROWS], F8 if FP8_H else BF16, tag="hT")
                row_slices = [(0, 480, 0), (480, 480, 4), (960, 64, None)]
                for m in range(8):
                    msl = slice(m * P, (m + 1) * P)
                    for r0, nsz, sblk0 in row_slices:
                        pt = psB.tile([P, 512], F32, tag="l1")
                        rsl = slice(r0, r0 + nsz)
                        chunks = [(nw0x_s[:, ks, msl], xT_s[:, ks, rsl]) for ks in range(4)]
                        chunks.append((nw0a_s[:, msl], actT_s[:, rsl]))
                        if sblk0 is not None:
                            chunks += [(nw0s_s[:, ks, msl],
                                        sT_v[:, ks, sblk0:sblk0 + 4, 0:NODES_BLK])
                                       for ks in range(8)]
                        for ci, (lhs, rhs) in enumerate(chunks):
                            nc.tensor.matmul(pt[:, 0:nsz], lhs, rhs,
                                             start=(ci == 0), stop=(ci == len(chunks) - 1))
                        nc.scalar.activation(hT[:, m, rsl], pt[:, 0:nsz], AF.Relu,
                                             bias=nb0_t[:, m:m + 1])

                # ---- node layer 2 (row-major) + LN + relu -> z2, dma-tp ----
                z2T = na.tile([P, 8, N_ROWS], BF16, tag="z2T")
                for rt in range(8):
                    pt = psA.tile([P, H], F32, tag="mm")
                    if FP8_H:
                        for kp in range(4):
                            lhs = hT[:, 2 * kp:2 * kp + 2, rt * P:(rt + 1) * P]
                            for half in (0, 512):
                                nc.tensor.matmul(pt[:, half:half + 512], lhs,
                                                 nw1_s[:, 2 * kp:2 * kp + 2, half:half + 512],
                                                 start=(kp == 0), stop=(kp == 3), perf_mode=DR)
                    else:
                        for ks in range(8):
                            lhs = hT[:, ks, rt * P:(rt + 1) * P]
                            for half in (0, 512):
                                nc.tensor.matmul(pt[:, half:half + 512], lhs,
                                                 nw1_s[:, ks, half:half + 512],
                                                 start=(ks == 0), stop=(ks == 7))
                    h2b = nst.tile([P, H], F32, tag="h2b")
                    nc.vector.tensor_tensor(h2b[:], pt[:], nb1_b[:], ALU.add)
                    st6 = nst.tile([P, 12], F32, tag="st6")
                    nc.vector.bn_stats(st6[:, 0:6], h2b[:, 0:512])
                    nc.vector.bn_stats(st6[:, 6:12], h2b[:, 512:1024])
                    mv = nst.tile([P, 2], F32, tag="mv")
                    nc.vector.bn_aggr(mv[:], st6[:].rearrange("p (a b) -> p a b", b=6))
                    sc = nst.tile([P, 2], F32, tag="sc")
                    nc.scalar.activation(sc[:, 0:1], mv[:, 1:2],
                                         AF.Abs_reciprocal_sqrt, bias=eps_t[:])
                    nc.vector.tensor_scalar(sc[:, 1:2], mv[:, 0:1], sc[:, 0:1], -1.0,
                                            ALU.mult, ALU.mult)
                    z2 = nst.tile([P, H], BF16, tag="z2")
                    if trivial_affine_n:
                        nc.scalar.activation(z2[:], h2b[:], AF.Relu,
                                             bias=sc[:, 1:2], scale=sc[:, 0:1])
                    else:
                        zn = nst.tile([P, H], F32, tag="zn")
                        nc.scalar.activation(zn[:], h2b[:], AF.Identity,
                                             bias=sc[:, 1:2], scale=sc[:, 0:1])
                        nc.vector.tensor_tensor(zn[:], zn[:], ng_b[:], ALU.mult)
                        nc.vector.tensor_tensor(zn[:], zn[:], nbe_b[:], ALU.add)
                        nc.scalar.activation(z2[:], zn[:], AF.Relu)
                    nc.scalar.dma_start_transpose(z2T[:, :, rt * P:(rt + 1) * P], z2[:])

                # ---- node layer 3 + bias ----
                out_r = out[:].rearrange("(rt p) d -> p rt d", p=P)
                for rt in range(8):
                    pt = psB.tile([P, 512], F32, tag="l3")
                    for ks in range(8):
                        nc.tensor.matmul(pt[:, 0:D], z2T[:, ks, rt * P:(rt + 1) * P],
                                         nw2_s[:, ks, :], start=(ks == 0), stop=False)
                    nc.tensor.matmul(pt[:, 0:D], ones_row[:], nb2_s[:], start=False, stop=True)
                    outb = nst.tile([P, D], F32, tag="outb")
                    nc.scalar.activation(outb[:], pt[:, 0:D], AF.Identity)
                    nc.sync.dma_start(out_r[:, rt, :], outb[:])

    return nc


_PROG_CACHE = {}


def _get_program(trivial_e, trivial_n):
    key = (trivial_e, trivial_n, FP8_H)
    if key not in _PROG_CACHE:
        nc = _build_program(trivial_e, trivial_n)
        nc.finalize()
        _PROG_CACHE[key] = nc
    return _PROG_CACHE[key]


def _pkn(w, kt):
    """[K, N] -> [P, kt, N] (partition-major, SBUF-ready)."""
    return np.ascontiguousarray(w.reshape(kt, P, w.shape[1]).transpose(1, 0, 2))


def kernel(states, action, e_w0, e_b0, e_w1, e_b1, e_g, e_be, e_w2, e_b2,
           n_w0, n_b0, n_w1, n_b1, n_g, n_be, n_w2, n_b2):
    states = _f32(states)
    action = np.asarray(action).astype(np.int64)
    e_w0, e_b0, e_w1, e_b1 = _f32(e_w0), _f32(e_b0), _f32(e_w1), _f32(e_b1)
    e_g, e_be, e_w2, e_b2 = _f32(e_g), _f32(e_be), _f32(e_w2), _f32(e_b2)
    n_w0, n_b0, n_w1, n_b1 = _f32(n_w0), _f32(n_b0), _f32(n_w1), _f32(n_b1)
    n_g, n_be, n_w2, n_b2 = _f32(n_g), _f32(n_be), _f32(n_w2), _f32(n_b2)

    trivial_e = bool(np.all(e_g == 1.0) and np.all(e_be == 0.0))
    trivial_n = bool(np.all(n_g == 1.0) and np.all(n_be == 0.0))
    nc = _get_program(trivial_e, trivial_n)

    flat = states.reshape(-1, D)                        # [8192, 512]
    av = np.zeros((B, A_DIM * K), dtype=np.float32)
    av[np.arange(B), action] = 1.0
    av = av.reshape(-1, A_DIM)                          # [8192, 20]

    wab = e_w0[0:D] + e_w0[D:2 * D]                     # [512, 1024]
    w0c = e_w0[2 * D:3 * D]
    nw0x = n_w0[0:D]
    nw0a = n_w0[D:D + A_DIM]
    n_w0s_part = n_w0[D + A_DIM:]
    nw0s = e_w2 @ n_w0s_part                            # [1024, 1024]
    nw0a21 = np.concatenate([nw0a, (e_b2 @ n_w0s_part).reshape(1, H)], axis=0)

    amat = _build_amat()                                # [NCHUNK, 128, 128]
    amat_pkn = np.ascontiguousarray(amat.transpose(1, 0, 2))  # [P, NCHUNK, P]

    common = {
        "wab": _bf16(_pkn(wab, 4)), "w0c": _bf16(_pkn(w0c, 4)),
        "b0": _f32(e_b0.reshape(8, P).T), "w1": _f8(_pkn(e_w1, 8)),
        "b1": _f8(e_b1.reshape(1, H)),
        "amat": _f8(amat_pkn),
        "nw0x": _bf16(_pkn(nw0x, 4)), "nw0a": _bf16(nw0a21),
        "nw0s": _bf16(_pkn(nw0s, 8)), "nb0": _f32(n_b0.reshape(8, P).T),
        "nw1": (_f8 if FP8_H else _bf16)(_pkn(n_w1, 8)),
        "nb1": _f32(n_b1.reshape(1, H)),
        "nw2": _bf16(_pkn(n_w2, 8)), "nb2": _bf16(n_b2.reshape(1, D)),
    }
    if not trivial_e:
        common["e_g"] = _f32(e_g)
        common["e_be"] = _f32(e_be)
    if not trivial_n:
        common["n_g"] = _f32(n_g)
        common["n_be"] = _f32(n_be)

    in_maps = []
    row_idx = []
    for c in range(N_CORES):
        idx = np.concatenate([
            np.arange(c * EDGE_ROWS, (c + 1) * EDGE_ROWS),
            np.arange(NG * 15 + c * EXTRA_ROWS, NG * 15 + (c + 1) * EXTRA_ROWS),
        ])
        row_idx.append(idx)
        x_rows = flat[idx]                              # [1024, 512]
        xt = x_rows.T.reshape(4, P, N_ROWS).transpose(1, 0, 2)  # [P, 4, N]
        at = np.concatenate([av[idx].T, np.concatenate(
            [np.full((1, EDGE_ROWS), 14.0, np.float32),
             np.zeros((1, EXTRA_ROWS), np.float32)], axis=1)], axis=0)  # [21, 1024]
        m = dict(common)
        m["xT"] = _bf16(np.ascontiguousarray(xt))
        m["actT"] = _bf16(at)
        in_maps.append(m)

    res = run_bass_kernel_spmd(nc, in_maps, core_ids=list(range(N_CORES)))
    global LAST_RESULT
    LAST_RESULT = res

    out_full = np.empty((B * K, D), dtype=np.float32)
    for c in range(N_CORES):
        out_full[row_idx[c]] = flat[row_idx[c]] + res.results[c]["out"]
    return out_full.reshape(B, K, D)


# revision 17
# speedup vs baseline: 1.6488x; 1.0324x over previous
"""CSWM transition GNN kernel for 8 TRN2 NeuronCores.

Sharding: data-parallel over the 512 edge-groups (the quirky edge list is
block-diagonal over groups of 15 consecutive flat rows). Each core gets
64 groups (960 edge rows) + 64 of the 512 zero-agg tail rows = 1024 node
rows. No cross-core communication.

Host-side algebra:
  - cat(xi,xi,xj)@e_w0 = xi@(W0a+W0b) + xj@W0c          (per-node U,V)
  - final edge matmul commutes with scatter-add; W2 then folds into the
    node MLP first layer: nw0s = e_w2 @ n_w0[532:1556]
  - per-edge work: one 1024x1024 fp8 matmul + LayerNorm + relu

v2 structure:
  - packed-210 edge slots (no diagonal): slot (g,i,j') -> edge
    (i, (i+1+j') mod 15) via an overlapping-window AP on a duplicated V
  - r-build adds on gpsimd, relus split scalar/vector
  - all transposes via DMA xbar (dma_start_transpose), none on PE
  - weights stored pre-transposed in DRAM (contiguous DMA)
  - node L2 in fp8 DoubleRow (hT evicted as fp8)
"""

import numpy as np
import ml_dtypes
import bass_rust

import concourse.bass as bass
import concourse.mybir as mybir
import concourse.tile as tile
from concourse import bacc
from concourse.bass_utils import run_bass_kernel_spmd

BF16 = mybir.dt.bfloat16
F32 = mybir.dt.float32
F8 = mybir.dt.float8e4
DR = mybir.MatmulPerfMode.DoubleRow
AF = mybir.ActivationFunctionType
ALU = mybir.AluOpType

P = 128
D = 512            # embedding dim
H = 1024           # hidden dim
A_DIM = 20         # action dim
B = 512            # batch
K = 16             # objects
NG = 512           # total edge groups (block-diag over 15-row groups)
N_CORES = 8
G_CORE = NG // N_CORES          # 64 groups per core
EDGE_ROWS = G_CORE * 15         # 960
EXTRA_ROWS = (B * K - NG * 15) // N_CORES   # 64 zero-agg tail rows per core
N_ROWS = EDGE_ROWS + EXTRA_ROWS  # 1024 node rows per core
GB = 8                          # groups per aggregation block
NBLK = G_CORE // GB             # 8 blocks per core
E_BLK = GB * 210                # 1680 edges per block (diagonal-free)
NCHUNK = (E_BLK + P - 1) // P   # 14 chunks of 128 edge-slots
NODES_BLK = GB * 15             # 120
EPS = 1e-5
FP8_H = False                   # node hidden in fp8 -> L2 DoubleRow

# r-build engine split: adds on gpsimd (2-input tensor_tensor is ~2x DVE but
# runs on an otherwise idle engine); relu+fp8-cast must NOT go on gpsimd
# (its quantizing tensor_scalar path measured ~25us per op).
ADD_ENG = ['g', 'g', 'g', 'g', 'g', 'g', 'g', 'g']
RELU_ENG = ['s', 's', 's', 's', 's', 'v', 'v', 'v']


def _bf16(x):
    return np.ascontiguousarray(np.asarray(x, dtype=np.float32).astype(ml_dtypes.bfloat16))


def _f8(x):
    return np.ascontiguousarray(np.asarray(x, dtype=np.float32).astype(ml_dtypes.float8_e4m3))


def _f32(x):
    return np.ascontiguousarray(np.asarray(x, dtype=np.float32))


def _ap_window(full_ap, dims, extra_offset):
    """Raw AP with explicit (step, count) dims (supports overlap/broadcast)."""
    c = full_ap.copy()
    c.ap = bass_rust.VecI64Pair(dims)
    c.offset = c.offset + extra_offset
    return c


def _build_amat():
    """[NCHUNK, 128, 128] 0/1: slot g*210+i*14+j' -> node g*15+i (col),
    chunk-local rows; padding rows/cols zero."""
    a = np.zeros((NCHUNK * P, P), dtype=np.float32)
    for s in range(E_BLK):
        g, rem = divmod(s, 210)
        i = rem // 14
        a[s, g * 15 + i] = 1.0
    return a.reshape(NCHUNK, P, P)


def _build_program(trivial_affine_e: bool, trivial_affine_n: bool):
    nc = bacc.Bacc("TRN2", target_bir_lowering=False, debug=False)

    def din(name, shape, dt):
        return nc.declare_dram_parameter(name, list(shape), dt, isOutput=False)

    # all weight layouts are SBUF-ready: [P, k, n] contiguous
    xT = din("xT", (P, 4, N_ROWS), BF16)
    actT = din("actT", (A_DIM + 1, N_ROWS), BF16)
    wab = din("wab", (P, 4, H), BF16)
    w0c = din("w0c", (P, 4, H), BF16)
    b0 = din("b0", (P, 8), F32)
    w1 = din("w1", (P, 8, H), F8)
    b1 = din("b1", (1, H), F8)
    amat = din("amat", (P, NCHUNK, P), F8)
    nw0x = din("nw0x", (P, 4, H), BF16)
    nw0a = din("nw0a", (A_DIM + 1, H), BF16)
    nw0s = din("nw0s", (P, 8, H), BF16)
    nb0 = din("nb0", (P, 8), F32)
    nw1 = din("nw1", (P, 8, H), F8 if FP8_H else BF16)
    nb1 = din("nb1", (1, H), F32)
    nw2 = din("nw2", (P, 8, D), BF16)
    nb2 = din("nb2", (1, D), BF16)
    if not trivial_affine_e:
        e_g = din("e_g", (H,), F32)
        e_be = din("e_be", (H,), F32)
    if not trivial_affine_n:
        n_g = din("n_g", (H,), F32)
        n_be = din("n_be", (H,), F32)

    out = nc.declare_dram_parameter("out", [N_ROWS, D], F32, isOutput=True)

    with tile.TileContext(nc) as tc:
        with tc.tile_pool(name="const", bufs=1) as cpool:
            xT_s = cpool.tile([P, 4, N_ROWS], BF16)
            actT_s = cpool.tile([A_DIM + 1, N_ROWS], BF16)
            ones_row = cpool.tile([1, P], BF16)
            nc.vector.memset(ones_row[:], 1.0)
            eps_t = cpool.tile([P, 1], F32)
            nc.vector.memset(eps_t[:], EPS)
            # sT: aggregated-hidden, feature-major, block-slotted:
            # sT[p, k, blk*128 + node] = s[blk nodes' row, k*128+p]
            sT = cpool.tile([P, 8, NBLK * P], BF16)
            # s_blk: node-major aggregated hidden per block (dma-tp source);
            # rows 120..127 zeroed once (the dma transpose reads all 128).
            s_blks = [cpool.tile([P, H], BF16, tag=f"sblk{b}", name=f"s_blk{b}")
                      for b in range(NBLK)]
            for b in range(NBLK):
                nc.vector.memset(s_blks[b][96:P, :], 0.0)

            # ================= EDGE PHASE =================
            with (
                tc.tile_pool(name="ew", bufs=1) as ew,
                tc.tile_pool(name="uv", bufs=1) as uvp,
                tc.tile_pool(name="rp", bufs=3) as rp,
                tc.tile_pool(name="rb", bufs=8) as rbp,
                tc.tile_pool(name="zp", bufs=6) as zp,
                tc.tile_pool(name="st", bufs=4) as stp,
                tc.tile_pool(name="ps", bufs=3, space="PSUM") as ps,
                tc.tile_pool(name="pa", bufs=1, space="PSUM") as pa,
            ):
                wab_s = ew.tile([P, 4, H], BF16)
                w0c_s = ew.tile([P, 4, H], BF16)
                b0_t = ew.tile([P, 8], F32)
                nc.sync.dma_start(b0_t[:], b0[:])
                for ks in range(4):
                    nc.sync.dma_start(wab_s[:, ks, :], wab[:, ks, :])
                    nc.sync.dma_start(xT_s[:, ks, :], xT[:, ks, :])
                nc.sync.dma_start(actT_s[:], actT[:])
                for ks in range(4):
                    nc.sync.dma_start(w0c_s[:, ks, :], w0c[:, ks, :])
                w1_s = ew.tile([P, 8, H], F8)
                nc.sync.dma_start(w1_s[:], w1[:])
                amat_s = ew.tile([P, NCHUNK, P], F8)
                nc.sync.dma_start(amat_s[:], amat[:])
                b1_r = ew.tile([1, H], F8)
                nc.sync.dma_start(b1_r[:], b1[:])
                ones8 = ew.tile([1, P], F8)
                nc.vector.memset(ones8[:], 1.0)
                if not trivial_affine_e:
                    eg_b = ew.tile([P, H], F32)
                    nc.sync.dma_start(eg_b[:], e_g[None, :].to_broadcast((P, H)))
                    ebe_b = ew.tile([P, H], F32)
                    nc.sync.dma_start(ebe_b[:], e_be[None, :].to_broadcast((P, H)))

                # ---- U = x@(W0a+W0b)+b0 (feat-major), V duplicated [g]15+15 ----
                u_s = uvp.tile([P, 8, EDGE_ROWS], BF16, tag="u")
                v2_s = uvp.tile([P, 8, 2 * EDGE_ROWS], BF16, tag="v2")
                r_tiles = {}
                rb_tiles = {}

                def r_add(blk, fs):
                    """rb = U[i] + V[j] for block blk, feature-chunk fs."""
                    rb = rbp.tile([P, E_BLK], BF16, tag="rb", name=f"rb{blk}_{fs}")
                    rb_tiles[(blk, fs)] = rb
                    uap = _ap_window(
                        u_s[:], [[8 * EDGE_ROWS, P], [15, GB], [1, 15], [0, 14]],
                        fs * EDGE_ROWS + blk * NODES_BLK)
                    vap = _ap_window(
                        v2_s[:], [[16 * EDGE_ROWS, P], [30, GB], [1, 15], [1, 14]],
                        fs * 2 * EDGE_ROWS + blk * 2 * NODES_BLK + 1)
                    rb_o = rb[:].rearrange("p (g i j) -> p g i j", i=15, j=14)
                    # block 0 builds on vector (idle during U/V); later blocks
                    # on gpsimd with a full block period of lead time
                    eng = nc.vector if blk == 0 else nc.gpsimd
                    eng.tensor_tensor(rb_o, uap, vap, ALU.add)

                def r_relu(blk, fs):
                    rt = r_tiles[blk % 3]
                    rb = rb_tiles.pop((blk, fs))
                    e = RELU_ENG[fs]
                    if e == 's':
                        nc.scalar.activation(rt[:, fs, :], rb[:], AF.Relu)
                    elif e == 'v':
                        nc.vector.tensor_scalar_max(rt[:, fs, :], rb[:], 0.0)
                    else:
                        nc.gpsimd.tensor_scalar_max(rt[:, fs, :], rb[:], 0.0)

                for m in range(8):
                    for dst, wt, bias in ((u_s, wab_s, True), (v2_s, w0c_s, False)):
                        pt = ps.tile([P, H], F32, tag="mm")
                        for half, ncols in ((0, 512), (512, EDGE_ROWS - 512)):
                            for ks in range(4):
                                nc.tensor.matmul(
                                    pt[:, half:half + ncols],
                                    wt[:, ks, m * P:(m + 1) * P],
                                    xT_s[:, ks, half:half + ncols],
                                    start=(ks == 0), stop=(ks == 3),
                                )
                        if bias:
                            nc.scalar.activation(
                                dst[:, m, :], pt[:, :EDGE_ROWS], AF.Identity,
                                bias=b0_t[:, m:m + 1])
                        else:
                            # duplicated V: v2[g*30 + t] = v2[g*30+15+t] = V[g*15+t]
                            dvo = dst[:, m, :].rearrange("p (g t) -> p g t", t=30)
                            src = pt[:, :EDGE_ROWS].rearrange("p (g t) -> p g t", t=15)
                            nc.scalar.activation(dvo[:, :, 0:15], src, AF.Identity)
                            nc.vector.tensor_scalar_add(dvo[:, :, 15:30], src, 0.0)
                    if m < 3:
                        r_tiles[m] = rp.tile([P, 8, E_BLK], F8, tag="r", name=f"r_t{m}")
                    # build r for block 0 as soon as fs-chunk m of U/V lands
                    r_add(0, m)
                    r_relu(0, m)

                # block 1's adds start as soon as U/V is complete
                for fs in range(8):
                    r_add(1, fs)

                # ---- per-block: edge matmul + LN + aggregate ----
                def emit_agg_pair(pagg, cp, zpair):
                    lhs = amat_s[:, 2 * cp:2 * cp + 2, 0:NODES_BLK]
                    for half in (0, 512):
                        nc.tensor.matmul(pagg[:, half:half + 512], lhs,
                                         zpair[:, :, half:half + 512],
                                         start=(cp == 0), stop=(cp == NCHUNK // 2 - 1),
                                         perf_mode=DR)

                for blk in range(NBLK):
                    nxt = blk + 1
                    rt = r_tiles[blk % 3]
                    pagg = pa.tile([NODES_BLK, H], F32, tag="agg")
                    z_pairs = []
                    for et in range(NCHUNK):
                        m_sz = min(P, E_BLK - et * P)
                        pt = ps.tile([P, H], F32, tag="mm")
                        for kp in range(4):
                            lhs = rt[:, 2 * kp:2 * kp + 2, et * P:et * P + m_sz]
                            for half in (0, 512):
                                nc.tensor.matmul(pt[:m_sz, half:half + 512], lhs,
                                                 w1_s[:, 2 * kp:2 * kp + 2, half:half + 512],
                                                 start=(kp == 0), stop=False, perf_mode=DR)
                        for half in (0, 512):
                            nc.tensor.matmul(pt[:m_sz, half:half + 512], ones8[:, :m_sz],
                                             b1_r[:, half:half + 512], start=False, stop=True)

                        if et % 2 == 0:
                            z_pair = zp.tile([P, 2, H], F8, tag="z")
                            z_pairs.append(z_pair)
                        z_t = z_pairs[et // 2][:, et % 2, :]
                        if m_sz < P:
                            nc.vector.memset(z_pairs[et // 2][:, et % 2, :], 0.0)

                        # next block's adds all at chunk 0: gpsimd gets a
                        # full block period of lead time for its 8 serial adds
                        if 1 < nxt < NBLK and et == 0:
                            for fs in range(8):
                                r_add(nxt, fs)

                        # LayerNorm stats on PSUM (pt already includes b1)
                        st6 = stp.tile([P, 12], F32, tag="st6")
                        nc.vector.bn_stats(st6[:m_sz, 0:6], pt[:m_sz, 0:512])
                        nc.vector.bn_stats(st6[:m_sz, 6:12], pt[:m_sz, 512:1024])
                        mv = stp.tile([P, 2], F32, tag="mv")
                        nc.vector.bn_aggr(mv[:m_sz], st6[:m_sz].rearrange("p (a b) -> p a b", b=6))
                        sc = stp.tile([P, 2], F32, tag="sc")
                        nc.scalar.activation(sc[:m_sz, 0:1], mv[:m_sz, 1:2],
                                             AF.Abs_reciprocal_sqrt, bias=eps_t[:m_sz])
                        nc.vector.tensor_scalar(sc[:m_sz, 1:2], mv[:m_sz, 0:1],
                                                sc[:m_sz, 0:1], -1.0,
                                                ALU.mult, ALU.mult)
                        if trivial_affine_e:
                            nc.scalar.activation(z_t[:m_sz], pt[:m_sz], AF.Relu,
                                                 bias=sc[:m_sz, 1:2], scale=sc[:m_sz, 0:1])
                        else:
                            zn = stp.tile([P, H], F32, tag="zn")
                            nc.scalar.activation(zn[:m_sz], pt[:m_sz], AF.Identity,
                                                 bias=sc[:m_sz, 1:2], scale=sc[:m_sz, 0:1])
                            nc.vector.tensor_tensor(zn[:m_sz], zn[:m_sz], eg_b[:m_sz], ALU.mult)
                            nc.vector.tensor_tensor(zn[:m_sz], zn[:m_sz], ebe_b[:m_sz], ALU.add)
                            nc.scalar.activation(z_t[:m_sz], zn[:m_sz], AF.Relu)

                        if nxt < NBLK and 2 <= et < 10:
                            r_relu(nxt, et - 2)

                        # aggregation trails the LN pipeline by one pair
                        if et % 2 == 1 and et >= 3:
                            emit_agg_pair(pagg, (et - 3) // 2, z_pairs[(et - 3) // 2])
                    emit_agg_pair(pagg, NCHUNK // 2 - 1, z_pairs[NCHUNK // 2 - 1])

                    # evict aggregated block; rows 120..127 are zeros
                    nc.scalar.activation(s_blks[blk][0:NODES_BLK, :], pagg[:], AF.Identity)
                    nc.scalar.dma_start_transpose(
                        sT[:, :, blk * P:(blk + 1) * P], s_blks[blk][:])

            # ================= NODE PHASE =================
            with (
                tc.tile_pool(name="nw", bufs=1) as nw,
                tc.tile_pool(name="nact", bufs=1) as na,
                tc.tile_pool(name="nst", bufs=3) as nst,
                tc.tile_pool(name="psA", bufs=2, space="PSUM") as psA,
                tc.tile_pool(name="psB", bufs=2, space="PSUM") as psB,
            ):
                nw0x_s = nw.tile([P, 4, H], BF16)
                nc.sync.dma_start(nw0x_s[:], nw0x[:])
                nw0a_s = nw.tile([A_DIM + 1, H], BF16)
                nc.sync.dma_start(nw0a_s[:], nw0a[:])
                nw0s_s = nw.tile([P, 8, H], BF16)
                nc.sync.dma_start(nw0s_s[:], nw0s[:])
                nw1_s = nw.tile([P, 8, H], F8 if FP8_H else BF16)
                nc.sync.dma_start(nw1_s[:], nw1[:])
                nw2_s = nw.tile([P, 8, D], BF16)
                nc.sync.dma_start(nw2_s[:], nw2[:])
                nb0_t = nw.tile([P, 8], F32)
                nc.sync.dma_start(nb0_t[:], nb0[:])
                nb1_b = nw.tile([P, H], F32)
                nc.sync.dma_start(nb1_b[:], nb1[:].to_broadcast((P, H)))
                nb2_s = nw.tile([1, D], BF16)
                nc.sync.dma_start(nb2_s[:], nb2[:])
                if not trivial_affine_n:
                    ng_b = nw.tile([P, H], F32)
                    nc.sync.dma_start(ng_b[:], n_g[None, :].to_broadcast((P, H)))
                    nbe_b = nw.tile([P, H], F32)
                    nc.sync.dma_start(nbe_b[:], n_be[None, :].to_broadcast((P, H)))

                sT_v = sT[:].rearrange("p k (b n) -> p k b n", n=P)

                # ---- node layer 1 -> hT (feat-major, relu+bias in evict) ----
                hT = na.tile([P, 8, N_ROWS], F8 if FP8_H else BF16, tag="hT")
                row_slices = [(0, 480, 0), (480, 480, 4), (960, 64, None)]
                for m in range(8):
                    msl = slice(m * P, (m + 1) * P)
                    for r0, nsz, sblk0 in row_slices:
                        pt = psB.tile([P, 512], F32, tag="l1")
                        rsl = slice(r0, r0 + nsz)
                        chunks = [(nw0x_s[:, ks, msl], xT_s[:, ks, rsl]) for ks in range(4)]
                        chunks.append((nw0a_s[:, msl], actT_s[:, rsl]))
                        if sblk0 is not None:
                            chunks += [(nw0s_s[:, ks, msl],
                                        sT_v[:, ks, sblk0:sblk0 + 4, 0:NODES_BLK])
                                       for ks in range(8)]
                        for ci, (lhs, rhs) in enumerate(chunks):
                            nc.tensor.matmul(pt[:, 0:nsz], lhs, rhs,
                                             start=(ci == 0), stop=(ci == len(chunks) - 1))
                        nc.scalar.activation(hT[:, m, rsl], pt[:, 0:nsz], AF.Relu,
                                             bias=nb0_t[:, m:m + 1])

                # ---- node layer 2 (row-major) + LN + relu -> z2, dma-tp ----
                z2T = na.tile([P, 8, N_ROWS], BF16, tag="z2T")
                for rt in range(8):
                    pt = psA.tile([P, H], F32, tag="mm")
                    if FP8_H:
                        for kp in range(4):
                            lhs = hT[:, 2 * kp:2 * kp + 2, rt * P:(rt + 1) * P]
                            for half in (0, 512):
                                nc.tensor.matmul(pt[:, half:half + 512], lhs,
                                                 nw1_s[:, 2 * kp:2 * kp + 2, half:half + 512],
                                                 start=(kp == 0), stop=(kp == 3), perf_mode=DR)
                    else:
                        for ks in range(8):
                            lhs = hT[:, ks, rt * P:(rt + 1) * P]
                            for half in (0, 512):
                                nc.tensor.matmul(pt[:, half:half + 512], lhs,
                                                 nw1_s[:, ks, half:half + 512],
                                                 start=(ks == 0), stop=(ks == 7))
                    h2b = nst.tile([P, H], F32, tag="h2b")
                    nc.vector.tensor_tensor(h2b[:], pt[:], nb1_b[:], ALU.add)
                    st6 = nst.tile([P, 12], F32, tag="st6")
                    nc.vector.bn_stats(st6[:, 0:6], h2b[:, 0:512])
                    nc.vector.bn_stats(st6[:, 6:12], h2b[:, 512:1024])
                    mv = nst.tile([P, 2], F32, tag="mv")
                    nc.vector.bn_aggr(mv[:], st6[:].rearrange("p (a b) -> p a b", b=6))
                    sc = nst.tile([P, 2], F32, tag="sc")
                    nc.scalar.activation(sc[:, 0:1], mv[:, 1:2],
                                         AF.Abs_reciprocal_sqrt, bias=eps_t[:])
                    nc.vector.tensor_scalar(sc[:, 1:2], mv[:, 0:1], sc[:, 0:1], -1.0,
                                            ALU.mult, ALU.mult)
                    z2 = nst.tile([P, H], BF16, tag="z2")
                    if trivial_affine_n:
                        nc.scalar.activation(z2[:], h2b[:], AF.Relu,
                                             bias=sc[:, 1:2], scale=sc[:, 0:1])
                    else:
                        zn = nst.tile([P, H], F32, tag="zn")
                        nc.scalar.activation(zn[:], h2b[:], AF.Identity,
                                             bias=sc[:, 1:2], scale=sc[:, 0:1])
                        nc.vector.tensor_tensor(zn[:], zn[:], ng_b[:], ALU.mult)
                        nc.vector.tensor_tensor(zn[:], zn[:], nbe_b[:], ALU.add)
                        nc.scalar.activation(z2[:], zn[:], AF.Relu)
                    nc.scalar.dma_start_transpose(z2T[:, :, rt * P:(rt + 1) * P], z2[:])

                # ---- node layer 3 + bias ----
                out_r = out[:].rearrange("(rt p) d -> p rt d", p=P)
                for rt in range(8):
                    pt = psB.tile([P, 512], F32, tag="l3")
                    for ks in range(8):
                        nc.tensor.matmul(pt[:, 0:D], z2T[:, ks, rt * P:(rt + 1) * P],
                                         nw2_s[:, ks, :], start=(ks == 0), stop=False)
                    nc.tensor.matmul(pt[:, 0:D], ones_row[:], nb2_s[:], start=False, stop=True)
                    outb = nst.tile([P, D], F32, tag="outb")
                    nc.scalar.activation(outb[:], pt[:, 0:D], AF.Identity)
                    nc.sync.dma_start(out_r[:, rt, :], outb[:])

    return nc


_PROG_CACHE = {}


def _get_program(trivial_e, trivial_n):
    key = (trivial_e, trivial_n, FP8_H)
    if key not in _PROG_CACHE:
        nc = _build_program(trivial_e, trivial_n)
        nc.finalize()
        _PROG_CACHE[key] = nc
    return _PROG_CACHE[key]


def _pkn(w, kt):
    """[K, N] -> [P, kt, N] (partition-major, SBUF-ready)."""
    return np.ascontiguousarray(w.reshape(kt, P, w.shape[1]).transpose(1, 0, 2))


def kernel(states, action, e_w0, e_b0, e_w1, e_b1, e_g, e_be, e_w2, e_b2,
           n_w0, n_b0, n_w1, n_b1, n_g, n_be, n_w2, n_b2):
    states = _f32(states)
    action = np.asarray(action).astype(np.int64)
    e_w0, e_b0, e_w1, e_b1 = _f32(e_w0), _f32(e_b0), _f32(e_w1), _f32(e_b1)
    e_g, e_be, e_w2, e_b2 = _f32(e_g), _f32(e_be), _f32(e_w2), _f32(e_b2)
    n_w0, n_b0, n_w1, n_b1 = _f32(n_w0), _f32(n_b0), _f32(n_w1), _f32(n_b1)
    n_g, n_be, n_w2, n_b2 = _f32(n_g), _f32(n_be), _f32(n_w2), _f32(n_b2)

    trivial_e = bool(np.all(e_g == 1.0) and np.all(e_be == 0.0))
    trivial_n = bool(np.all(n_g == 1.0) and np.all(n_be == 0.0))
    nc = _get_program(trivial_e, trivial_n)

    flat = states.reshape(-1, D)                        # [8192, 512]
    av = np.zeros((B, A_DIM * K), dtype=np.float32)
    av[np.arange(B), action] = 1.0
    av = av.reshape(-1, A_DIM)                          # [8192, 20]

    wab = e_w0[0:D] + e_w0[D:2 * D]                     # [512, 1024]
    w0c = e_w0[2 * D:3 * D]
    nw0x = n_w0[0:D]
    nw0a = n_w0[D:D + A_DIM]
    n_w0s_part = n_w0[D + A_DIM:]
    nw0s = e_w2 @ n_w0s_part                            # [1024, 1024]
    nw0a21 = np.concatenate([nw0a, (e_b2 @ n_w0s_part).reshape(1, H)], axis=0)

    amat = _build_amat()                                # [NCHUNK, 128, 128]
    amat_pkn = np.ascontiguousarray(amat.transpose(1, 0, 2))  # [P, NCHUNK, P]

    common = {
        "wab": _bf16(_pkn(wab, 4)), "w0c": _bf16(_pkn(w0c, 4)),
        "b0": _f32(e_b0.reshape(8, P).T), "w1": _f8(_pkn(e_w1, 8)),
        "b1": _f8(e_b1.reshape(1, H)),
        "amat": _f8(amat_pkn),
        "nw0x": _bf16(_pkn(nw0x, 4)), "nw0a": _bf16(nw0a21),
        "nw0s": _bf16(_pkn(nw0s, 8)), "nb0": _f32(n_b0.reshape(8, P).T),
        "nw1": (_f8 if FP8_H else _bf16)(_pkn(n_w1, 8)),
        "nb1": _f32(n_b1.reshape(1, H)),
        "nw2": _bf16(_pkn(n_w2, 8)), "nb2": _bf16(n_b2.reshape(1, D)),
    }
    if not trivial_e:
        common["e_g"] = _f32(e_g)
        common["e_be"] = _f32(e_be)
    if not trivial_n:
        common["n_g"] = _f32(n_g)
        common["n_be"] = _f32(n_be)

    in_maps = []
    row_idx = []
    for c in range(N_CORES):
        idx = np.concatenate([
            np.arange(c * EDGE_ROWS, (c + 1) * EDGE_ROWS),
            np.arange(NG * 15 + c * EXTRA_ROWS, NG * 15 + (c + 1) * EXTRA_ROWS),
        ])
        row_idx.append(idx)
        x_rows = flat[idx]                              # [1024, 512]
        xt = x_rows.T.reshape(4, P, N_ROWS).transpose(1, 0, 2)  # [P, 4, N]
        at = np.concatenate([av[idx].T, np.concatenate(
            [np.full((1, EDGE_ROWS), 14.0, np.float32),
             np.zeros((1, EXTRA_ROWS), np.float32)], axis=1)], axis=0)  # [21, 1024]
        m = dict(common)
        m["xT"] = _bf16(np.ascontiguousarray(xt))
        m["actT"] = _bf16(at)
        in_maps.append(m)

    res = run_bass_kernel_spmd(nc, in_maps, core_ids=list(range(N_CORES)))
    global LAST_RESULT
    LAST_RESULT = res

    out_full = np.empty((B * K, D), dtype=np.float32)
    for c in range(N_CORES):
        out_full[row_idx[c]] = flat[row_idx[c]] + res.results[c]["out"]
    return out_full.reshape(B, K, D)


# revision 18
# speedup vs baseline: 1.6580x; 1.0056x over previous
"""CSWM transition GNN kernel for 8 TRN2 NeuronCores.

Sharding: data-parallel over the 512 edge-groups (the quirky edge list is
block-diagonal over groups of 15 consecutive flat rows). Each core gets
64 groups (960 edge rows) + 64 of the 512 zero-agg tail rows = 1024 node
rows. No cross-core communication.

Host-side algebra:
  - cat(xi,xi,xj)@e_w0 = xi@(W0a+W0b) + xj@W0c          (per-node U,V)
  - final edge matmul commutes with scatter-add; W2 then folds into the
    node MLP first layer: nw0s = e_w2 @ n_w0[532:1556]
  - per-edge work: one 1024x1024 fp8 matmul + LayerNorm + relu

v2 structure:
  - packed-210 edge slots (no diagonal): slot (g,i,j') -> edge
    (i, (i+1+j') mod 15) via an overlapping-window AP on a duplicated V
  - r-build adds on gpsimd, relus split scalar/vector
  - all transposes via DMA xbar (dma_start_transpose), none on PE
  - weights stored pre-transposed in DRAM (contiguous DMA)
  - node L2 in fp8 DoubleRow (hT evicted as fp8)
"""

import numpy as np
import ml_dtypes
import bass_rust

import concourse.bass as bass
import concourse.mybir as mybir
import concourse.tile as tile
from concourse import bacc
from concourse.bass_utils import run_bass_kernel_spmd

BF16 = mybir.dt.bfloat16
F32 = mybir.dt.float32
F8 = mybir.dt.float8e4
DR = mybir.MatmulPerfMode.DoubleRow
AF = mybir.ActivationFunctionType
ALU = mybir.AluOpType

P = 128
D = 512            # embedding dim
H = 1024           # hidden dim
A_DIM = 20         # action dim
B = 512            # batch
K = 16             # objects
NG = 512           # total edge groups (block-diag over 15-row groups)
N_CORES = 8
G_CORE = NG // N_CORES          # 64 groups per core
EDGE_ROWS = G_CORE * 15         # 960
EXTRA_ROWS = (B * K - NG * 15) // N_CORES   # 64 zero-agg tail rows per core
N_ROWS = EDGE_ROWS + EXTRA_ROWS  # 1024 node rows per core
GB = 8                          # groups per aggregation block
NBLK = G_CORE // GB             # 8 blocks per core
E_BLK = GB * 210                # 1680 edges per block (diagonal-free)
NCHUNK = (E_BLK + P - 1) // P   # 14 chunks of 128 edge-slots
NODES_BLK = GB * 15             # 120
EPS = 1e-5
FP8_H = False                   # node hidden in fp8 -> L2 DoubleRow

# r-build engine split: adds on gpsimd (2-input tensor_tensor is ~2x DVE but
# runs on an otherwise idle engine); relu+fp8-cast must NOT go on gpsimd
# (its quantizing tensor_scalar path measured ~25us per op).
ADD_ENG = ['g', 'g', 'g', 'g', 'g', 'g', 'g', 'g']
RELU_ENG = ['s', 's', 's', 's', 'v', 'v', 'v', 'v']


def _bf16(x):
    return np.ascontiguousarray(np.asarray(x, dtype=np.float32).astype(ml_dtypes.bfloat16))


def _f8(x):
    return np.ascontiguousarray(np.asarray(x, dtype=np.float32).astype(ml_dtypes.float8_e4m3))


def _f32(x):
    return np.ascontiguousarray(np.asarray(x, dtype=np.float32))


def _ap_window(full_ap, dims, extra_offset):
    """Raw AP with explicit (step, count) dims (supports overlap/broadcast)."""
    c = full_ap.copy()
    c.ap = bass_rust.VecI64Pair(dims)
    c.offset = c.offset + extra_offset
    return c


def _build_amat():
    """[NCHUNK, 128, 128] 0/1: slot g*210+i*14+j' -> node g*15+i (col),
    chunk-local rows; padding rows/cols zero."""
    a = np.zeros((NCHUNK * P, P), dtype=np.float32)
    for s in range(E_BLK):
        g, rem = divmod(s, 210)
        i = rem // 14
        a[s, g * 15 + i] = 1.0
    return a.reshape(NCHUNK, P, P)


def _build_program(trivial_affine_e: bool, trivial_affine_n: bool):
    nc = bacc.Bacc("TRN2", target_bir_lowering=False, debug=False)

    def din(name, shape, dt):
        return nc.declare_dram_parameter(name, list(shape), dt, isOutput=False)

    # all weight layouts are SBUF-ready: [P, k, n] contiguous
    xT = din("xT", (P, 4, N_ROWS), BF16)
    actT = din("actT", (A_DIM + 1, N_ROWS), BF16)
    wab = din("wab", (P, 4, H), BF16)
    w0c = din("w0c", (P, 4, H), BF16)
    b0 = din("b0", (P, 8), F32)
    w1 = din("w1", (P, 8, H), F8)
    b1 = din("b1", (1, H), F8)
    amat = din("amat", (P, NCHUNK, P), F8)
    nw0x = din("nw0x", (P, 4, H), BF16)
    nw0a = din("nw0a", (A_DIM + 1, H), BF16)
    nw0s = din("nw0s", (P, 8, H), BF16)
    nb0 = din("nb0", (P, 8), F32)
    nw1 = din("nw1", (P, 8, H), F8 if FP8_H else BF16)
    nb1 = din("nb1", (1, H), F32)
    nw2 = din("nw2", (P, 8, D), BF16)
    nb2 = din("nb2", (1, D), BF16)
    if not trivial_affine_e:
        e_g = din("e_g", (H,), F32)
        e_be = din("e_be", (H,), F32)
    if not trivial_affine_n:
        n_g = din("n_g", (H,), F32)
        n_be = din("n_be", (H,), F32)

    out = nc.declare_dram_parameter("out", [N_ROWS, D], F32, isOutput=True)

    with tile.TileContext(nc) as tc:
        with tc.tile_pool(name="const", bufs=1) as cpool:
            xT_s = cpool.tile([P, 4, N_ROWS], BF16)
            actT_s = cpool.tile([A_DIM + 1, N_ROWS], BF16)
            ones_row = cpool.tile([1, P], BF16)
            nc.vector.memset(ones_row[:], 1.0)
            eps_t = cpool.tile([P, 1], F32)
            nc.vector.memset(eps_t[:], EPS)
            # sT: aggregated-hidden, feature-major, block-slotted:
            # sT[p, k, blk*128 + node] = s[blk nodes' row, k*128+p]
            sT = cpool.tile([P, 8, NBLK * P], BF16)
            # s_blk: node-major aggregated hidden per block (dma-tp source);
            # rows 120..127 zeroed once (the dma transpose reads all 128).
            s_blks = [cpool.tile([P, H], BF16, tag=f"sblk{b}", name=f"s_blk{b}")
                      for b in range(NBLK)]
            for b in range(NBLK):
                nc.vector.memset(s_blks[b][96:P, :], 0.0)

            # ================= EDGE PHASE =================
            with (
                tc.tile_pool(name="ew", bufs=1) as ew,
                tc.tile_pool(name="uv", bufs=1) as uvp,
                tc.tile_pool(name="rp", bufs=3) as rp,
                tc.tile_pool(name="rb", bufs=8) as rbp,
                tc.tile_pool(name="zp", bufs=6) as zp,
                tc.tile_pool(name="st", bufs=4) as stp,
                tc.tile_pool(name="ps", bufs=3, space="PSUM") as ps,
                tc.tile_pool(name="pa", bufs=1, space="PSUM") as pa,
            ):
                wab_s = ew.tile([P, 4, H], BF16)
                w0c_s = ew.tile([P, 4, H], BF16)
                b0_t = ew.tile([P, 8], F32)
                nc.sync.dma_start(b0_t[:], b0[:])
                for ks in range(4):
                    nc.sync.dma_start(wab_s[:, ks, :], wab[:, ks, :])
                    nc.sync.dma_start(xT_s[:, ks, :], xT[:, ks, :])
                nc.sync.dma_start(actT_s[:], actT[:])
                for ks in range(4):
                    nc.sync.dma_start(w0c_s[:, ks, :], w0c[:, ks, :])
                w1_s = ew.tile([P, 8, H], F8)
                nc.sync.dma_start(w1_s[:], w1[:])
                amat_s = ew.tile([P, NCHUNK, P], F8)
                nc.sync.dma_start(amat_s[:], amat[:])
                b1_r = ew.tile([1, H], F8)
                nc.sync.dma_start(b1_r[:], b1[:])
                ones8 = ew.tile([1, P], F8)
                nc.vector.memset(ones8[:], 1.0)
                if not trivial_affine_e:
                    eg_b = ew.tile([P, H], F32)
                    nc.sync.dma_start(eg_b[:], e_g[None, :].to_broadcast((P, H)))
                    ebe_b = ew.tile([P, H], F32)
                    nc.sync.dma_start(ebe_b[:], e_be[None, :].to_broadcast((P, H)))

                # ---- U = x@(W0a+W0b)+b0 (feat-major), V duplicated [g]15+15 ----
                u_s = uvp.tile([P, 8, EDGE_ROWS], BF16, tag="u")
                v2_s = uvp.tile([P, 8, 2 * EDGE_ROWS], BF16, tag="v2")
                r_tiles = {}
                rb_tiles = {}

                def r_add(blk, fs):
                    """rb = U[i] + V[j] for block blk, feature-chunk fs."""
                    rb = rbp.tile([P, E_BLK], BF16, tag="rb", name=f"rb{blk}_{fs}")
                    rb_tiles[(blk, fs)] = rb
                    uap = _ap_window(
                        u_s[:], [[8 * EDGE_ROWS, P], [15, GB], [1, 15], [0, 14]],
                        fs * EDGE_ROWS + blk * NODES_BLK)
                    vap = _ap_window(
                        v2_s[:], [[16 * EDGE_ROWS, P], [30, GB], [1, 15], [1, 14]],
                        fs * 2 * EDGE_ROWS + blk * 2 * NODES_BLK + 1)
                    rb_o = rb[:].rearrange("p (g i j) -> p g i j", i=15, j=14)
                    # block 0 builds on vector (idle during U/V); later blocks
                    # on gpsimd with a full block period of lead time
                    eng = nc.vector if blk == 0 else nc.gpsimd
                    eng.tensor_tensor(rb_o, uap, vap, ALU.add)

                def r_relu(blk, fs):
                    rt = r_tiles[blk % 3]
                    rb = rb_tiles.pop((blk, fs))
                    e = RELU_ENG[fs]
                    if e == 's':
                        nc.scalar.activation(rt[:, fs, :], rb[:], AF.Relu)
                    elif e == 'v':
                        nc.vector.tensor_scalar_max(rt[:, fs, :], rb[:], 0.0)
                    else:
                        nc.gpsimd.tensor_scalar_max(rt[:, fs, :], rb[:], 0.0)

                for m in range(8):
                    for dst, wt, bias in ((u_s, wab_s, True), (v2_s, w0c_s, False)):
                        pt = ps.tile([P, H], F32, tag="mm")
                        for half, ncols in ((0, 512), (512, EDGE_ROWS - 512)):
                            for ks in range(4):
                                nc.tensor.matmul(
                                    pt[:, half:half + ncols],
                                    wt[:, ks, m * P:(m + 1) * P],
                                    xT_s[:, ks, half:half + ncols],
                                    start=(ks == 0), stop=(ks == 3),
                                )
                        if bias:
                            nc.scalar.activation(
                                dst[:, m, :], pt[:, :EDGE_ROWS], AF.Identity,
                                bias=b0_t[:, m:m + 1])
                        else:
                            # duplicated V: v2[g*30 + t] = v2[g*30+15+t] = V[g*15+t]
                            dvo = dst[:, m, :].rearrange("p (g t) -> p g t", t=30)
                            src = pt[:, :EDGE_ROWS].rearrange("p (g t) -> p g t", t=15)
                            nc.scalar.activation(dvo[:, :, 0:15], src, AF.Identity)
                            nc.vector.tensor_scalar_add(dvo[:, :, 15:30], src, 0.0)
                    if m < 3:
                        r_tiles[m] = rp.tile([P, 8, E_BLK], F8, tag="r", name=f"r_t{m}")
                    # build r for block 0 as soon as fs-chunk m of U/V lands
                    r_add(0, m)
                    r_relu(0, m)

                # block 1's adds start as soon as U/V is complete
                for fs in range(8):
                    r_add(1, fs)

                # ---- per-block: edge matmul + LN + aggregate ----
                def emit_agg_pair(pagg, cp, zpair):
                    lhs = amat_s[:, 2 * cp:2 * cp + 2, 0:NODES_BLK]
                    for half in (0, 512):
                        nc.tensor.matmul(pagg[:, half:half + 512], lhs,
                                         zpair[:, :, half:half + 512],
                                         start=(cp == 0), stop=(cp == NCHUNK // 2 - 1),
                                         perf_mode=DR)

                for blk in range(NBLK):
                    nxt = blk + 1
                    rt = r_tiles[blk % 3]
                    pagg = pa.tile([NODES_BLK, H], F32, tag="agg")
                    z_pairs = []
                    for et in range(NCHUNK):
                        m_sz = min(P, E_BLK - et * P)
                        pt = ps.tile([P, H], F32, tag="mm")
                        for kp in range(4):
                            lhs = rt[:, 2 * kp:2 * kp + 2, et * P:et * P + m_sz]
                            for half in (0, 512):
                                nc.tensor.matmul(pt[:m_sz, half:half + 512], lhs,
                                                 w1_s[:, 2 * kp:2 * kp + 2, half:half + 512],
                                                 start=(kp == 0), stop=False, perf_mode=DR)
                        for half in (0, 512):
                            nc.tensor.matmul(pt[:m_sz, half:half + 512], ones8[:, :m_sz],
                                             b1_r[:, half:half + 512], start=False, stop=True)

                        if et % 2 == 0:
                            z_pair = zp.tile([P, 2, H], F8, tag="z")
                            z_pairs.append(z_pair)
                        z_t = z_pairs[et // 2][:, et % 2, :]
                        if m_sz < P:
                            nc.vector.memset(z_pairs[et // 2][:, et % 2, :], 0.0)

                        # next block's adds all at chunk 0: gpsimd gets a
                        # full block period of lead time for its 8 serial adds
                        if 1 < nxt < NBLK and et == 0:
                            for fs in range(8):
                                r_add(nxt, fs)

                        # LayerNorm stats on PSUM (pt already includes b1)
                        st6 = stp.tile([P, 12], F32, tag="st6")
                        nc.vector.bn_stats(st6[:m_sz, 0:6], pt[:m_sz, 0:512])
                        nc.vector.bn_stats(st6[:m_sz, 6:12], pt[:m_sz, 512:1024])
                        mv = stp.tile([P, 2], F32, tag="mv")
                        nc.vector.bn_aggr(mv[:m_sz], st6[:m_sz].rearrange("p (a b) -> p a b", b=6))
                        sc = stp.tile([P, 2], F32, tag="sc")
                        nc.scalar.activation(sc[:m_sz, 0:1], mv[:m_sz, 1:2],
                                             AF.Abs_reciprocal_sqrt, bias=eps_t[:m_sz])
                        nc.vector.tensor_scalar(sc[:m_sz, 1:2], mv[:m_sz, 0:1],
                                                sc[:m_sz, 0:1], -1.0,
                                                ALU.mult, ALU.mult)
                        if trivial_affine_e:
                            nc.scalar.activation(z_t[:m_sz], pt[:m_sz], AF.Relu,
                                                 bias=sc[:m_sz, 1:2], scale=sc[:m_sz, 0:1])
                        else:
                            zn = stp.tile([P, H], F32, tag="zn")
                            nc.scalar.activation(zn[:m_sz], pt[:m_sz], AF.Identity,
                                                 bias=sc[:m_sz, 1:2], scale=sc[:m_sz, 0:1])
                            nc.vector.tensor_tensor(zn[:m_sz], zn[:m_sz], eg_b[:m_sz], ALU.mult)
                            nc.vector.tensor_tensor(zn[:m_sz], zn[:m_sz], ebe_b[:m_sz], ALU.add)
                            nc.scalar.activation(z_t[:m_sz], zn[:m_sz], AF.Relu)

                        if nxt < NBLK and 2 <= et < 10:
                            r_relu(nxt, et - 2)

                        # aggregation trails the LN pipeline by one pair
                        if et % 2 == 1 and et >= 3:
                            emit_agg_pair(pagg, (et - 3) // 2, z_pairs[(et - 3) // 2])
                    emit_agg_pair(pagg, NCHUNK // 2 - 1, z_pairs[NCHUNK // 2 - 1])

                    # evict aggregated block; rows 120..127 are zeros
                    nc.scalar.activation(s_blks[blk][0:NODES_BLK, :], pagg[:], AF.Identity)
                    nc.scalar.dma_start_transpose(
                        sT[:, :, blk * P:(blk + 1) * P], s_blks[blk][:])

            # ================= NODE PHASE =================
            with (
                tc.tile_pool(name="nw", bufs=1) as nw,
                tc.tile_pool(name="nact", bufs=1) as na,
                tc.tile_pool(name="nst", bufs=3) as nst,
                tc.tile_pool(name="psA", bufs=2, space="PSUM") as psA,
                tc.tile_pool(name="psB", bufs=2, space="PSUM") as psB,
            ):
                nw0x_s = nw.tile([P, 4, H], BF16)
                nc.sync.dma_start(nw0x_s[:], nw0x[:])
                nw0a_s = nw.tile([A_DIM + 1, H], BF16)
                nc.sync.dma_start(nw0a_s[:], nw0a[:])
                nw0s_s = nw.tile([P, 8, H], BF16)
                nc.sync.dma_start(nw0s_s[:], nw0s[:])
                nw1_s = nw.tile([P, 8, H], F8 if FP8_H else BF16)
                nc.sync.dma_start(nw1_s[:], nw1[:])
                nw2_s = nw.tile([P, 8, D], BF16)
                nc.sync.dma_start(nw2_s[:], nw2[:])
                nb0_t = nw.tile([P, 8], F32)
                nc.sync.dma_start(nb0_t[:], nb0[:])
                nb1_b = nw.tile([P, H], F32)
                nc.sync.dma_start(nb1_b[:], nb1[:].to_broadcast((P, H)))
                nb2_s = nw.tile([1, D], BF16)
                nc.sync.dma_start(nb2_s[:], nb2[:])
                if not trivial_affine_n:
                    ng_b = nw.tile([P, H], F32)
                    nc.sync.dma_start(ng_b[:], n_g[None, :].to_broadcast((P, H)))
                    nbe_b = nw.tile([P, H], F32)
                    nc.sync.dma_start(nbe_b[:], n_be[None, :].to_broadcast((P, H)))

                sT_v = sT[:].rearrange("p k (b n) -> p k b n", n=P)

                # ---- node layer 1 -> hT (feat-major, relu+bias in evict) ----
                hT = na.tile([P, 8, N_ROWS], F8 if FP8_H else BF16, tag="hT")
                row_slices = [(0, 480, 0), (480, 480, 4), (960, 64, None)]
                for m in range(8):
                    msl = slice(m * P, (m + 1) * P)
                    for r0, nsz, sblk0 in row_slices:
                        pt = psB.tile([P, 512], F32, tag="l1")
                        rsl = slice(r0, r0 + nsz)
                        chunks = [(nw0x_s[:, ks, msl], xT_s[:, ks, rsl]) for ks in range(4)]
                        chunks.append((nw0a_s[:, msl], actT_s[:, rsl]))
                        if sblk0 is not None:
                            chunks += [(nw0s_s[:, ks, msl],
                                        sT_v[:, ks, sblk0:sblk0 + 4, 0:NODES_BLK])
                                       for ks in range(8)]
                        for ci, (lhs, rhs) in enumerate(chunks):
                            nc.tensor.matmul(pt[:, 0:nsz], lhs, rhs,
                                             start=(ci == 0), stop=(ci == len(chunks) - 1))
                        nc.scalar.activation(hT[:, m, rsl], pt[:, 0:nsz], AF.Relu,
                                             bias=nb0_t[:, m:m + 1])

                # ---- node layer 2 (row-major) + LN + relu -> z2, dma-tp ----
                z2T = na.tile([P, 8, N_ROWS], BF16, tag="z2T")
                for rt in range(8):
                    pt = psA.tile([P, H], F32, tag="mm")
                    if FP8_H:
                        for kp in range(4):
                            lhs = hT[:, 2 * kp:2 * kp + 2, rt * P:(rt + 1) * P]
                            for half in (0, 512):
                                nc.tensor.matmul(pt[:, half:half + 512], lhs,
                                                 nw1_s[:, 2 * kp:2 * kp + 2, half:half + 512],
                                                 start=(kp == 0), stop=(kp == 3), perf_mode=DR)
                    else:
                        for ks in range(8):
                            lhs = hT[:, ks, rt * P:(rt + 1) * P]
                            for half in (0, 512):
                                nc.tensor.matmul(pt[:, half:half + 512], lhs,
                                                 nw1_s[:, ks, half:half + 512],
                                                 start=(ks == 0), stop=(ks == 7))
                    h2b = nst.tile([P, H], F32, tag="h2b")
                    nc.vector.tensor_tensor(h2b[:], pt[:], nb1_b[:], ALU.add)
                    st6 = nst.tile([P, 12], F32, tag="st6")
                    nc.vector.bn_stats(st6[:, 0:6], h2b[:, 0:512])
                    nc.vector.bn_stats(st6[:, 6:12], h2b[:, 512:1024])
                    mv = nst.tile([P, 2], F32, tag="mv")
                    nc.vector.bn_aggr(mv[:], st6[:].rearrange("p (a b) -> p a b", b=6))
                    sc = nst.tile([P, 2], F32, tag="sc")
                    nc.scalar.activation(sc[:, 0:1], mv[:, 1:2],
                                         AF.Abs_reciprocal_sqrt, bias=eps_t[:])
                    nc.vector.tensor_scalar(sc[:, 1:2], mv[:, 0:1], sc[:, 0:1], -1.0,
                                            ALU.mult, ALU.mult)
                    z2 = nst.tile([P, H], BF16, tag="z2")
                    if trivial_affine_n:
                        nc.scalar.activation(z2[:], h2b[:], AF.Relu,
                                             bias=sc[:, 1:2], scale=sc[:, 0:1])
                    else:
                        zn = nst.tile([P, H], F32, tag="zn")
                        nc.scalar.activation(zn[:], h2b[:], AF.Identity,
                                             bias=sc[:, 1:2], scale=sc[:, 0:1])
                        nc.vector.tensor_tensor(zn[:], zn[:], ng_b[:], ALU.mult)
                        nc.vector.tensor_tensor(zn[:], zn[:], nbe_b[:], ALU.add)
                        nc.scalar.activation(z2[:], zn[:], AF.Relu)
                    nc.sync.dma_start_transpose(z2T[:, :, rt * P:(rt + 1) * P], z2[:])

                # ---- node layer 3 + bias ----
                out_r = out[:].rearrange("(rt p) d -> p rt d", p=P)
                for rt in range(8):
                    pt = psB.tile([P, 512], F32, tag="l3")
                    for ks in range(8):
                        nc.tensor.matmul(pt[:, 0:D], z2T[:, ks, rt * P:(rt + 1) * P],
                                         nw2_s[:, ks, :], start=(ks == 0), stop=False)
                    nc.tensor.matmul(pt[:, 0:D], ones_row[:], nb2_s[:], start=False, stop=True)
                    outb = nst.tile([P, D], F32, tag="outb")
                    nc.scalar.activation(outb[:], pt[:, 0:D], AF.Identity)
                    nc.sync.dma_start(out_r[:, rt, :], outb[:])

    return nc


_PROG_CACHE = {}


def _get_program(trivial_e, trivial_n):
    key = (trivial_e, trivial_n, FP8_H)
    if key not in _PROG_CACHE:
        nc = _build_program(trivial_e, trivial_n)
        nc.finalize()
        _PROG_CACHE[key] = nc
    return _PROG_CACHE[key]


def _pkn(w, kt):
    """[K, N] -> [P, kt, N] (partition-major, SBUF-ready)."""
    return np.ascontiguousarray(w.reshape(kt, P, w.shape[1]).transpose(1, 0, 2))


def kernel(states, action, e_w0, e_b0, e_w1, e_b1, e_g, e_be, e_w2, e_b2,
           n_w0, n_b0, n_w1, n_b1, n_g, n_be, n_w2, n_b2):
    states = _f32(states)
    action = np.asarray(action).astype(np.int64)
    e_w0, e_b0, e_w1, e_b1 = _f32(e_w0), _f32(e_b0), _f32(e_w1), _f32(e_b1)
    e_g, e_be, e_w2, e_b2 = _f32(e_g), _f32(e_be), _f32(e_w2), _f32(e_b2)
    n_w0, n_b0, n_w1, n_b1 = _f32(n_w0), _f32(n_b0), _f32(n_w1), _f32(n_b1)
    n_g, n_be, n_w2, n_b2 = _f32(n_g), _f32(n_be), _f32(n_w2), _f32(n_b2)

    trivial_e = bool(np.all(e_g == 1.0) and np.all(e_be == 0.0))
    trivial_n = bool(np.all(n_g == 1.0) and np.all(n_be == 0.0))
    nc = _get_program(trivial_e, trivial_n)

    flat = states.reshape(-1, D)                        # [8192, 512]
    av = np.zeros((B, A_DIM * K), dtype=np.float32)
    av[np.arange(B), action] = 1.0
    av = av.reshape(-1, A_DIM)                          # [8192, 20]

    wab = e_w0[0:D] + e_w0[D:2 * D]                     # [512, 1024]
    w0c = e_w0[2 * D:3 * D]
    nw0x = n_w0[0:D]
    nw0a = n_w0[D:D + A_DIM]
    n_w0s_part = n_w0[D + A_DIM:]
    nw0s = e_w2 @ n_w0s_part                            # [1024, 1024]
    nw0a21 = np.concatenate([nw0a, (e_b2 @ n_w0s_part).reshape(1, H)], axis=0)

    amat = _build_amat()                                # [NCHUNK, 128, 128]
    amat_pkn = np.ascontiguousarray(amat.transpose(1, 0, 2))  # [P, NCHUNK, P]

    common = {
        "wab": _bf16(_pkn(wab, 4)), "w0c": _bf16(_pkn(w0c, 4)),
        "b0": _f32(e_b0.reshape(8, P).T), "w1": _f8(_pkn(e_w1, 8)),
        "b1": _f8(e_b1.reshape(1, H)),
        "amat": _f8(amat_pkn),
        "nw0x": _bf16(_pkn(nw0x, 4)), "nw0a": _bf16(nw0a21),
        "nw0s": _bf16(_pkn(nw0s, 8)), "nb0": _f32(n_b0.reshape(8, P).T),
        "nw1": (_f8 if FP8_H else _bf16)(_pkn(n_w1, 8)),
        "nb1": _f32(n_b1.reshape(1, H)),
        "nw2": _bf16(_pkn(n_w2, 8)), "nb2": _bf16(n_b2.reshape(1, D)),
    }
    if not trivial_e:
        common["e_g"] = _f32(e_g)
        common["e_be"] = _f32(e_be)
    if not trivial_n:
        common["n_g"] = _f32(n_g)
        common["n_be"] = _f32(n_be)

    in_maps = []
    row_idx = []
    for c in range(N_CORES):
        idx = np.concatenate([
            np.arange(c * EDGE_ROWS, (c + 1) * EDGE_ROWS),
            np.arange(NG * 15 + c * EXTRA_ROWS, NG * 15 + (c + 1) * EXTRA_ROWS),
        ])
        row_idx.append(idx)
        x_rows = flat[idx]                              # [1024, 512]
        xt = x_rows.T.reshape(4, P, N_ROWS).transpose(1, 0, 2)  # [P, 4, N]
        at = np.concatenate([av[idx].T, np.concatenate(
            [np.full((1, EDGE_ROWS), 14.0, np.float32),
             np.zeros((1, EXTRA_ROWS), np.float32)], axis=1)], axis=0)  # [21, 1024]
        m = dict(common)
        m["xT"] = _bf16(np.ascontiguousarray(xt))
        m["actT"] = _bf16(at)
        in_maps.append(m)

    res = run_bass_kernel_spmd(nc, in_maps, core_ids=list(range(N_CORES)))
    global LAST_RESULT
    LAST_RESULT = res

    out_full = np.empty((B * K, D), dtype=np.float32)
    for c in range(N_CORES):
        out_full[row_idx[c]] = flat[row_idx[c]] + res.results[c]["out"]
    return out_full.reshape(B, K, D)
